# revision 19
# baseline (speedup 1.0000x reference)
"""Trainium2 Bass kernel for the topk_masking memory-module problem.

Computation (reference semantics):
  For each of n=16 memory slots l:
    h = LeakyReLU_{slope_l}(q @ W1[l] + b1[l])          # [b, L, d]
    x = tanh(h @ W2[l] + b2[l])                          # [b, L, d]
    logits = x @ Wg[l] + bg[l]                           # [b, L]
    w = softmax(logits over L); top8 (values+indices)
    combined[b] = sum_k w_topk[k] * x[b, idx_k]          # [b, d]
  out[b, l, :] = normalize(combined over d)

Sharding: expert-parallel over the 16 memory slots -> 2 slots per core on
8 cores.  Each core runs the full [4, 4096, 512] query through its two
slots.  Device does everything except the final L2 normalize (host, cheap).

Device algorithm per core (pass A computes logits while discarding x; the
top-8 rows of x are recomputed in pass B from the gathered q rows):
  pass A: for b, for t (8 row-tiles of 512):
    hT = leaky(W1^T-chunks @ qT-tile + b1)   (transposed pipeline, d on
    xT = tanh(W2-chunks @ hT + b2)            partitions, rows on free)
    logits[2b+s, t*512:] = Wg . xT + bg      (PE matvec)
  per b: max8 + max_index give top-8 values+indices (paired, descending);
    softmax stats via Exp activation with accumulate; weights from values.
  pass B per (b, s): indirect-DMA gather the 8 q rows, recompute their x
    (tiny matmuls), then combined = x_sel^T @ w8 on PE; DMA to out[s,:,b].
"""

import numpy as np

import concourse.bass as bass
import concourse.bacc as bacc
import concourse.mybir as mybir
from concourse import bass_utils
from concourse.tile import TileContext

F32 = mybir.dt.float32
F32R = mybir.dt.float32r
U32 = mybir.dt.uint32
AF = mybir.ActivationFunctionType
ALU = mybir.AluOpType

B = 4
L = 4096
D = 512
N_MEM = 16
NCORES = 8
S = N_MEM // NCORES  # 2 slots per core
K = 8
T = L // 512  # 8 row-tiles per batch
PD = 128     # partition dim
KC = D // PD  # 4 contraction chunks

# The heavy pipeline runs the PE in float32r (fp32 operands, 1 cycle/row vs
# 4 for plain float32; slightly reduced multiply precision).  Top-k selection
# is sensitive to logit error, so this choice is validated against the
# reference in test.py on both CPU- and axon-generated datasets.
_PROGRAM_CACHE = {}


def _build_program():
    if "nc" in _PROGRAM_CACHE:
        return _PROGRAM_CACHE["nc"]

    nc = bacc.Bacc("TRN2", debug=False, enable_asserts=False, num_devices=NCORES)

    qt = nc.dram_tensor("qt", [B, D, L], F32R, kind="ExternalInput").ap()
    qn = nc.dram_tensor("qn", [B, L, D], F32, kind="ExternalInput").ap()
    w1 = nc.dram_tensor("w1", [S, D, D], F32R, kind="ExternalInput").ap()
    w2 = nc.dram_tensor("w2", [S, D, D], F32R, kind="ExternalInput").ap()
    # small constants packed into two tensors (one DMA each) so consumers
    # carry few semaphore waits: misc128 = [b1t | sb1t | b2t | slopet],
    # misc8 = [b1rep | b2rep | slope8]
    misc128 = nc.dram_tensor("misc128", [PD, 3 * S * KC + S], F32,
                             kind="ExternalInput").ap()
    wgt = nc.dram_tensor("wgt", [PD, S * KC], F32R, kind="ExternalInput").ap()
    bgt = nc.dram_tensor("bgt", [1, S], F32, kind="ExternalInput").ap()
    misc8 = nc.dram_tensor("misc8", [K, 2 * S * D + S], F32,
                           kind="ExternalInput").ap()
    ident = nc.dram_tensor("ident", [PD, PD], F32, kind="ExternalInput").ap()
    out = nc.dram_tensor("out", [S, D, B], F32, kind="ExternalOutput").ap()

    qn_flat = qn.rearrange("b l d -> (b l) d")

    with TileContext(nc) as tc:
        with (
            tc.tile_pool(name="consts", bufs=1) as cpool,
            tc.tile_pool(name="weights", bufs=1) as wpool,
            tc.tile_pool(name="qtp", bufs=2) as qtpool,
            tc.tile_pool(name="ht", bufs=3) as htpool,
            tc.tile_pool(name="xt", bufs=3) as xtpool,
            tc.tile_pool(name="tmp", bufs=3) as tmppool,
            tc.tile_pool(name="logits", bufs=1) as lpool,
            tc.tile_pool(name="small", bufs=8) as spool,
            tc.tile_pool(name="expp", bufs=1) as epool,
            tc.tile_pool(name="selp", bufs=2) as selpool,
            tc.tile_pool(name="hps", bufs=2, space="PSUM") as hps_pool,
            tc.tile_pool(name="xps", bufs=2, space="PSUM") as xps_pool,
            tc.tile_pool(name="gps", bufs=1, space="PSUM") as gps_pool,
            tc.tile_pool(name="bps", bufs=2, space="PSUM") as bps_pool,
        ):
            # --- persistent constants / weights in SBUF ---
            misc128_sb = cpool.tile_from(misc128)
            wgt_sb = cpool.tile_from(wgt)
            bgt_sb = cpool.tile_from(bgt)
            misc8_sb = cpool.tile_from(misc8)
            ident_sb = cpool.tile_from(ident)
            # TensorScalarPtr (scalar-operand-from-AP) instructions can carry
            # only one sync wait, so scalar sources must be same-engine local:
            # stage DVE-consumed constants through a DVE copy and ACT-consumed
            # biases through an ACT copy.  After these copies each engine has
            # observed the const DMA sem once, so no later op re-waits on it.
            misc128L = cpool.tile([PD, 3 * S * KC + S], F32, name="misc128L")
            nc.vector.tensor_copy(out=misc128L[:], in_=misc128_sb[:])
            misc8L = cpool.tile([K, 2 * S * D + S], F32, name="misc8L")
            nc.vector.tensor_copy(out=misc8L[:], in_=misc8_sb[:])
            b2tA = cpool.tile([PD, S * KC], F32, name="b2tA")
            nc.scalar.copy(out=b2tA[:], in_=misc128_sb[:, 2 * S * KC:3 * S * KC])
            bgtA = cpool.tile([1, S], F32, name="bgtA")
            nc.scalar.copy(out=bgtA[:], in_=bgt_sb[:])
            b1t_sb = misc128L[:, 0:S * KC]
            sb1t_sb = misc128L[:, S * KC:2 * S * KC]
            b2t_sb = b2tA[:]
            slopet_sb = misc128L[:, 3 * S * KC:3 * S * KC + S]
            b1rep_sb = misc8L[:, 0:S * D]
            b2rep_sb = misc8L[:, S * D:2 * S * D]
            slope8_sb = misc8L[:, 2 * S * D:2 * S * D + S]

            # weight tiles: w1_sb[s][kc] is [128, 512] rows d_in of chunk kc
            w1_sb = [[wpool.tile([PD, D], F32R, name=f"w1sb_{s}_{kc}", tag=f"w1_{s}_{kc}")
                      for kc in range(KC)] for s in range(S)]
            w2_sb = [[wpool.tile([PD, D], F32R, name=f"w2sb_{s}_{kc}", tag=f"w2_{s}_{kc}")
                      for kc in range(KC)] for s in range(S)]
            # order: everything mm1(s=0) needs first, so the PE can start
            # ~10us earlier; w2/s1 weights arrive while mm1(s0) runs
            for s in range(S):
                for kc in range(KC):
                    nc.gpsimd.dma_start(out=w1_sb[s][kc][:], in_=w1[s, kc * PD:(kc + 1) * PD, :])
            for s in range(S):
                for kc in range(KC):
                    nc.gpsimd.dma_start(out=w2_sb[s][kc][:], in_=w2[s, kc * PD:(kc + 1) * PD, :])

            # ---------------- PASS A + per-b topk / pass B ----------------
            for b in range(B):
                lrow_b = lpool.tile([S, L], F32, tag="lrow", bufs=2)
                for t in range(T):
                    # load qT tile: [128, kc x 512] (d on partitions, rows free)
                    qt_tile = qtpool.tile([PD, KC * 512], F32R, tag="qt")
                    src = qt[b, :, t * 512:(t + 1) * 512].rearrange(
                        "(kc p) r -> p kc r", p=PD)
                    dst = qt_tile[:].rearrange("p (kc r) -> p kc r", r=512)
                    nc.sync.dma_start(out=dst, in_=src)

                    ht_tiles = []
                    # mm1 + leaky for both slots (interleaved for PE density)
                    for s in range(S):
                        ht = htpool.tile([PD, KC * 512], F32R, tag="ht")
                        ht_tiles.append(ht)
                        for mc in range(KC):
                            h_ps = hps_pool.tile([PD, 512], F32, tag="hps")
                            for kc in range(KC):
                                nc.tensor.matmul(
                                    h_ps[:],
                                    lhsT=w1_sb[s][kc][:, mc * PD:(mc + 1) * PD],
                                    rhs=qt_tile[:, kc * 512:(kc + 1) * 512],
                                    start=(kc == 0), stop=(kc == KC - 1),
                                )
                            # leaky: u = h + b1; out = max(slope*u, u).
                            # The add uses tensor_tensor with a stride-0
                            # broadcast AP (TensorScalarPtr carries at most
                            # one sync wait; this op needs PE+DVE waits).
                            col = s * KC + mc
                            v = tmppool.tile([PD, 512], F32, tag="v")
                            nc.vector.tensor_tensor(
                                out=v[:], in0=h_ps[:],
                                in1=b1t_sb[:, col:col + 1].to_broadcast([PD, 512]),
                                op=ALU.add,
                            )
                            nc.vector.scalar_tensor_tensor(
                                out=ht[:, mc * 512:(mc + 1) * 512],
                                in0=v[:],
                                scalar=slopet_sb[:, s:s + 1],
                                in1=v[:],
                                op0=ALU.mult, op1=ALU.max,
                            )
                    xt_tiles = []
                    for s in range(S):
                        ht = ht_tiles[s]
                        xt = xtpool.tile([PD, KC * 512], F32R, tag="xt")
                        xt_tiles.append(xt)
                        for mc in range(KC):
                            x_ps = xps_pool.tile([PD, 512], F32, tag="xps")
                            for kc in range(KC):
                                nc.tensor.matmul(
                                    x_ps[:],
                                    lhsT=w2_sb[s][kc][:, mc * PD:(mc + 1) * PD],
                                    rhs=ht[:, kc * 512:(kc + 1) * 512],
                                    start=(kc == 0), stop=(kc == KC - 1),
                                )
                            col = s * KC + mc
                            nc.scalar.activation(
                                out=xt[:, mc * 512:(mc + 1) * 512], in_=x_ps[:],
                                func=AF.Tanh, bias=b2t_sb[:, col:col + 1],
                            )
                    # gate matvec per slot ([1, 512] psum each; PE out must
                    # start at partition 0). Engine ops can only address SBUF
                    # partitions 0/32/64/96, so stage the row at partition 0
                    # (with +bg) and DMA it into lrow_b's partition s.
                    for s in range(S):
                        g_ps = gps_pool.tile([1, 512], F32, tag="gps")
                        for kc in range(KC):
                            nc.tensor.matmul(
                                g_ps[:],
                                lhsT=wgt_sb[:, s * KC + kc:s * KC + kc + 1],
                                rhs=xt_tiles[s][:, kc * 512:(kc + 1) * 512],
                                start=(kc == 0), stop=(kc == KC - 1),
                            )
                        lstage = tmppool.tile([1, 512], F32, tag="lstage")
                        nc.scalar.activation(
                            out=lstage[:], in_=g_ps[:], func=AF.Identity,
                            bias=bgtA[0:1, s:s + 1],
                        )
                        nc.sync.dma_start(
                            out=lrow_b[s:s + 1, t * 512:(t + 1) * 512],
                            in_=lstage[:])

                # ---- top-k + softmax stats for this b (pairs 2b, 2b+1) ----
                lrow = lrow_b[:]
                mx = spool.tile([S, K], F32, tag="mx")
                idx = spool.tile([S, K], U32, tag="idx")
                nc.vector.max(out=mx[:], in_=lrow)
                nc.vector.max_index(out=idx[:], in_max=mx[:], in_values=lrow)
                negvmax = spool.tile([S, 1], F32, tag="nvm")
                nc.vector.tensor_scalar_mul(negvmax[:], mx[:, 0:1], -1.0)
                expt = epool.tile([S, L], F32, tag="expt")
                zsum = spool.tile([S, 1], F32, tag="zsum")
                nc.scalar.activation(
                    out=expt[:], in_=lrow, func=AF.Exp,
                    bias=negvmax[:, 0:1], accum_out=zsum[:, 0:1],
                )
                recipz = spool.tile([S, 1], F32, tag="rz")
                nc.vector.reciprocal(recipz[:], zsum[:])
                # w8 = exp(mx - vmax) * recipz   (paired with idx by rank)
                w8e = spool.tile([S, K], F32, tag="w8e")
                nc.vector.tensor_scalar_add(w8e[:], mx[:], negvmax[:, 0:1])
                nc.scalar.activation(out=w8e[:], in_=w8e[:], func=AF.Exp)
                w8 = spool.tile([S, K], F32, tag="w8")
                nc.vector.tensor_tensor(
                    out=w8[:], in0=w8e[:],
                    in1=recipz[:, 0:1].to_broadcast([S, K]), op=ALU.mult)
                # global row index = idx + b*4096 (as f32; exact for < 2^24)
                idxf = spool.tile([S, K], F32, tag="idxf")
                nc.vector.tensor_copy(out=idxf[:], in_=idx[:])
                nc.vector.tensor_scalar_add(idxf[:], idxf[:], float(b * L))
                # transpose idxf and w8 to [K, S] (rank on partitions)
                iw_ps = bps_pool.tile([K, 2 * S], F32, tag="bps")
                nc.tensor.transpose(iw_ps[:, 0:S], idxf[:], ident_sb[0:S, 0:S])
                nc.tensor.transpose(iw_ps[:, S:2 * S], w8[:], ident_sb[0:S, 0:S])
                iw_sb = spool.tile([K, 2 * S], F32, tag="iwsb")
                nc.vector.tensor_copy(out=iw_sb[:], in_=iw_ps[:])
                idxu = spool.tile([K, S], U32, tag="idxu")
                nc.vector.tensor_copy(out=idxu[:], in_=iw_sb[:, 0:S])

                # ---------------- PASS B: recompute top-8 rows ----------------
                for s in range(S):
                    q_sel = selpool.tile([K, D], F32, tag="qsel")
                    nc.gpsimd.indirect_dma_start(
                        out=q_sel[:], out_offset=None,
                        in_=qn_flat,
                        in_offset=bass.IndirectOffsetOnAxis(ap=idxu[:, s:s + 1], axis=0),
                    )
                    # q_selT chunks [128, 8] per kc
                    qst = selpool.tile([PD, KC * K], F32R, tag="qst")
                    for kc in range(KC):
                        t_ps = bps_pool.tile([PD, K], F32, tag="bps")
                        nc.tensor.transpose(
                            t_ps[:], q_sel[:, kc * PD:(kc + 1) * PD], ident_sb[0:K, 0:K])
                        nc.vector.tensor_copy(out=qst[:, kc * K:(kc + 1) * K], in_=t_ps[:])
                    # mm1 for selected rows: [8, 512]
                    hsel_ps = bps_pool.tile([K, D], F32, tag="bps2", bufs=1)
                    for kc in range(KC):
                        nc.tensor.matmul(
                            hsel_ps[:],
                            lhsT=qst[:, kc * K:(kc + 1) * K],
                            rhs=w1_sb[s][kc][:],
                            start=(kc == 0), stop=(kc == KC - 1),
                        )
                    hsel = selpool.tile([K, D], F32, tag="hsel")
                    nc.vector.tensor_tensor(
                        out=hsel[:], in0=hsel_ps[:],
                        in1=b1rep_sb[:, s * D:(s + 1) * D], op=ALU.add)
                    nc.vector.scalar_tensor_tensor(
                        out=hsel[:], in0=hsel[:], scalar=slope8_sb[:, s:s + 1],
                        in1=hsel[:], op0=ALU.mult, op1=ALU.max)
                    # transpose hsel -> [128, 8] chunks
                    hst = selpool.tile([PD, KC * K], F32R, tag="hst")
                    for kc in range(KC):
                        t_ps = bps_pool.tile([PD, K], F32, tag="bps")
                        nc.tensor.transpose(
                            t_ps[:], hsel[:, kc * PD:(kc + 1) * PD], ident_sb[0:K, 0:K])
                        nc.vector.tensor_copy(out=hst[:, kc * K:(kc + 1) * K], in_=t_ps[:])
                    xsel_ps = bps_pool.tile([K, D], F32, tag="bps2", bufs=1)
                    for kc in range(KC):
                        nc.tensor.matmul(
                            xsel_ps[:],
                            lhsT=hst[:, kc * K:(kc + 1) * K],
                            rhs=w2_sb[s][kc][:],
                            start=(kc == 0), stop=(kc == KC - 1),
                        )
                    xsel = selpool.tile([K, D], F32, tag="xsel")
                    nc.vector.tensor_tensor(
                        out=xsel[:], in0=xsel_ps[:],
                        in1=b2rep_sb[:, s * D:(s + 1) * D], op=ALU.add)
                    nc.scalar.activation(out=xsel[:], in_=xsel[:], func=AF.Tanh)
                    # combined[d] = sum_k w8[k] * xsel[k, d] -> [128, 1] per chunk
                    comb_ps = bps_pool.tile([PD, KC], F32, tag="bps")
                    for mc in range(KC):
                        nc.tensor.matmul(
                            comb_ps[:, mc:mc + 1],
                            lhsT=xsel[:, mc * PD:(mc + 1) * PD],
                            rhs=iw_sb[:, S + s:S + s + 1],
                            start=True, stop=True,
                        )
                    comb_sb = selpool.tile([PD, KC], F32, tag="combsb")
                    nc.vector.tensor_copy(out=comb_sb[:], in_=comb_ps[:])
                    nc.sync.dma_start(
                        out=out[s, :, b].rearrange("(mc p) -> p mc", p=PD),
                        in_=comb_sb[:],
                    )

    nc.compile()  # Bacc passes: reg alloc, DCE, wait splitting (TRN2 1-wait rule)
    _PROGRAM_CACHE["nc"] = nc
    return nc


def _prep_in_maps(query, W1, b1, W2, b2, Wg, bg):
    query = np.ascontiguousarray(query, dtype=np.float32)
    qt = np.ascontiguousarray(query.transpose(0, 2, 1))
    slopes = (0.01 + 0.2 / N_MEM * np.arange(N_MEM, dtype=np.float32))
    ident = np.eye(PD, dtype=np.float32)
    in_maps = []
    for c in range(NCORES):
        sl = slice(S * c, S * (c + 1))
        b1c = np.asarray(b1[sl], np.float32)     # [S, D]
        b2c = np.asarray(b2[sl], np.float32)
        wgc = np.asarray(Wg[sl], np.float32)
        slc = slopes[sl]                          # [S]
        sb1c = slc[:, None] * b1c
        def t128(a):  # [S, D] -> [128, S*KC]
            return np.ascontiguousarray(
                a.reshape(S, KC, PD).transpose(2, 0, 1).reshape(PD, S * KC))
        misc128 = np.concatenate([
            t128(b1c), t128(sb1c), t128(b2c),
            np.broadcast_to(slc[None, :], (PD, S)),
        ], axis=1).astype(np.float32)
        misc8 = np.concatenate([
            np.broadcast_to(b1c.reshape(1, S * D), (K, S * D)),
            np.broadcast_to(b2c.reshape(1, S * D), (K, S * D)),
            np.broadcast_to(slc[None, :], (K, S)),
        ], axis=1).astype(np.float32)
        in_maps.append({
            "qt": qt,
            "qn": query,
            "w1": np.ascontiguousarray(W1[sl], np.float32),
            "w2": np.ascontiguousarray(W2[sl], np.float32),
            "misc128": np.ascontiguousarray(misc128),
            "wgt": t128(wgc),
            "bgt": np.asarray(bg[sl], np.float32).reshape(1, S),
            "misc8": np.ascontiguousarray(misc8),
            "ident": ident,
        })
    return in_maps


def kernel(query, W1, b1, W2, b2, Wg, bg, topk, **_ignored):
    assert int(topk) == K, f"kernel hardcodes topk={K}, got {topk}"
    nc = _build_program()
    in_maps = _prep_in_maps(
        np.asarray(query), np.asarray(W1), np.asarray(b1), np.asarray(W2),
        np.asarray(b2), np.asarray(Wg), np.asarray(bg))
    res = bass_utils.run_bass_kernel_spmd(nc, in_maps, core_ids=list(range(NCORES)))
    outs = res.results
    # outs[c]["out"] is [S, D, B]; assemble [B, N_MEM, D]
    m = np.empty((B, N_MEM, D), dtype=np.float32)
    for c in range(NCORES):
        oc = np.asarray(outs[c]["out"])  # [S, D, B]
        for s in range(S):
            m[:, S * c + s, :] = oc[s].T
    norm = np.maximum(np.linalg.norm(m.astype(np.float64), axis=-1, keepdims=True),
                      1e-12).astype(np.float32)
    return (m / norm).astype(np.float32)


# revision 21
# speedup vs baseline: 1.0223x; 1.0223x over previous
"""Trainium2 Bass kernel for the topk_masking memory-module problem.

Computation (reference semantics):
  For each of n=16 memory slots l:
    h = LeakyReLU_{slope_l}(q @ W1[l] + b1[l])          # [b, L, d]
    x = tanh(h @ W2[l] + b2[l])                          # [b, L, d]
    logits = x @ Wg[l] + bg[l]                           # [b, L]
    w = softmax(logits over L); top8 (values+indices)
    combined[b] = sum_k w_topk[k] * x[b, idx_k]          # [b, d]
  out[b, l, :] = normalize(combined over d)

Sharding: expert-parallel over the 16 memory slots -> 2 slots per core on
8 cores.  Each core runs the full [4, 4096, 512] query through its two
slots.  Device does everything except the final L2 normalize (host, cheap).

Device algorithm per core (pass A computes logits while discarding x; the
top-8 rows of x are recomputed in pass B from the gathered q rows):
  pass A: for b, for t (8 row-tiles of 512):
    hT = leaky(W1^T-chunks @ qT-tile + b1)   (transposed pipeline, d on
    xT = tanh(W2-chunks @ hT + b2)            partitions, rows on free)
    logits[2b+s, t*512:] = Wg . xT + bg      (PE matvec)
  per b: max8 + max_index give top-8 values+indices (paired, descending);
    softmax stats via Exp activation with accumulate; weights from values.
  pass B per (b, s): indirect-DMA gather the 8 q rows, recompute their x
    (tiny matmuls), then combined = x_sel^T @ w8 on PE; DMA to out[s,:,b].
"""

import numpy as np

import concourse.bass as bass
import concourse.bacc as bacc
import concourse.mybir as mybir
from concourse import bass_utils
from concourse.tile import TileContext

F32 = mybir.dt.float32
F32R = mybir.dt.float32r
U32 = mybir.dt.uint32
AF = mybir.ActivationFunctionType
ALU = mybir.AluOpType

B = 4
L = 4096
D = 512
N_MEM = 16
NCORES = 8
S = N_MEM // NCORES  # 2 slots per core
K = 8
T = L // 512  # 8 row-tiles per batch
PD = 128     # partition dim
KC = D // PD  # 4 contraction chunks

# The heavy pipeline runs the PE in float32r (fp32 operands, 1 cycle/row vs
# 4 for plain float32; slightly reduced multiply precision).  Top-k selection
# is sensitive to logit error, so this choice is validated against the
# reference in test.py on both CPU- and axon-generated datasets.
_PROGRAM_CACHE = {}


def _build_program():
    if "nc" in _PROGRAM_CACHE:
        return _PROGRAM_CACHE["nc"]

    nc = bacc.Bacc("TRN2", debug=False, enable_asserts=False, num_devices=NCORES)

    qt = nc.dram_tensor("qt", [B, D, L], F32R, kind="ExternalInput").ap()
    qn = nc.dram_tensor("qn", [B, L, D], F32, kind="ExternalInput").ap()
    w1 = nc.dram_tensor("w1", [S, D, D], F32R, kind="ExternalInput").ap()
    w2 = nc.dram_tensor("w2", [S, D, D], F32R, kind="ExternalInput").ap()
    # small constants packed into two tensors (one DMA each) so consumers
    # carry few semaphore waits: misc128 = [b1t | sb1t | b2t | slopet],
    # misc8 = [b1rep | b2rep | slope8]
    misc128 = nc.dram_tensor("misc128", [PD, 3 * S * KC + S], F32,
                             kind="ExternalInput").ap()
    wgt = nc.dram_tensor("wgt", [PD, S * KC], F32R, kind="ExternalInput").ap()
    bgt = nc.dram_tensor("bgt", [1, S], F32, kind="ExternalInput").ap()
    misc8 = nc.dram_tensor("misc8", [K, 2 * S * D + S], F32,
                           kind="ExternalInput").ap()
    ident = nc.dram_tensor("ident", [PD, PD], F32, kind="ExternalInput").ap()
    out = nc.dram_tensor("out", [S, D, B], F32, kind="ExternalOutput").ap()

    qn_flat = qn.rearrange("b l d -> (b l) d")

    with TileContext(nc) as tc:
        with (
            tc.tile_pool(name="consts", bufs=1) as cpool,
            tc.tile_pool(name="weights", bufs=1) as wpool,
            tc.tile_pool(name="qtp", bufs=2) as qtpool,
            tc.tile_pool(name="ht", bufs=3) as htpool,
            tc.tile_pool(name="xt", bufs=3) as xtpool,
            tc.tile_pool(name="tmp", bufs=3) as tmppool,
            tc.tile_pool(name="logits", bufs=1) as lpool,
            tc.tile_pool(name="small", bufs=8) as spool,
            tc.tile_pool(name="expp", bufs=1) as epool,
            tc.tile_pool(name="selp", bufs=2) as selpool,
            tc.tile_pool(name="hps", bufs=2, space="PSUM") as hps_pool,
            tc.tile_pool(name="xps", bufs=2, space="PSUM") as xps_pool,
            tc.tile_pool(name="gps", bufs=1, space="PSUM") as gps_pool,
            tc.tile_pool(name="bps", bufs=2, space="PSUM") as bps_pool,
        ):
            # --- persistent constants / weights in SBUF ---
            misc128_sb = cpool.tile_from(misc128, forced_dma_engine=mybir.EngineType.Pool)
            wgt_sb = cpool.tile_from(wgt, forced_dma_engine=mybir.EngineType.Pool)
            bgt_sb = cpool.tile_from(bgt, forced_dma_engine=mybir.EngineType.Pool)
            misc8_sb = cpool.tile_from(misc8, forced_dma_engine=mybir.EngineType.Pool)
            ident_sb = cpool.tile_from(ident, forced_dma_engine=mybir.EngineType.Pool)
            # TensorScalarPtr (scalar-operand-from-AP) instructions can carry
            # only one sync wait, so scalar sources must be same-engine local:
            # stage DVE-consumed constants through a DVE copy and ACT-consumed
            # biases through an ACT copy.  After these copies each engine has
            # observed the const DMA sem once, so no later op re-waits on it.
            misc128L = cpool.tile([PD, 3 * S * KC + S], F32, name="misc128L")
            nc.vector.tensor_copy(out=misc128L[:], in_=misc128_sb[:])
            misc8L = cpool.tile([K, 2 * S * D + S], F32, name="misc8L")
            nc.vector.tensor_copy(out=misc8L[:], in_=misc8_sb[:])
            m128A = cpool.tile([PD, 3 * S * KC], F32, name="m128A")
            nc.scalar.copy(out=m128A[:], in_=misc128_sb[:, 0:3 * S * KC])
            b1tA = m128A[:, 0:S * KC]
            b2tA = m128A[:, 2 * S * KC:3 * S * KC]
            bgtA = cpool.tile([1, S], F32, name="bgtA")
            nc.scalar.copy(out=bgtA[:], in_=bgt_sb[:])
            b1t_sb = misc128L[:, 0:S * KC]
            sb1t_sb = misc128L[:, S * KC:2 * S * KC]
            b2t_sb = b2tA[:]
            slopet_sb = misc128L[:, 3 * S * KC:3 * S * KC + S]
            b1rep_sb = misc8L[:, 0:S * D]
            b2rep_sb = misc8L[:, S * D:2 * S * D]
            slope8_sb = misc8L[:, 2 * S * D:2 * S * D + S]

            # weight tiles: w1_sb[s][kc] is [128, 512] rows d_in of chunk kc
            w1_sb = [[wpool.tile([PD, D], F32R, name=f"w1sb_{s}_{kc}", tag=f"w1_{s}_{kc}")
                      for kc in range(KC)] for s in range(S)]
            w2_sb = [[wpool.tile([PD, D], F32R, name=f"w2sb_{s}_{kc}", tag=f"w2_{s}_{kc}")
                      for kc in range(KC)] for s in range(S)]
            # order: everything mm1(s=0) needs first, so the PE can start
            # ~10us earlier; w2/s1 weights arrive while mm1(s0) runs
            for s in range(S):
                for kc in range(KC):
                    nc.gpsimd.dma_start(out=w1_sb[s][kc][:], in_=w1[s, kc * PD:(kc + 1) * PD, :])
            for s in range(S):
                for kc in range(KC):
                    nc.gpsimd.dma_start(out=w2_sb[s][kc][:], in_=w2[s, kc * PD:(kc + 1) * PD, :])

            # ---------------- PASS A + per-b topk / pass B ----------------
            def topk_passB(b, lrow_b):
                # ---- top-k + softmax stats for this b (pairs 2b, 2b+1) ----
                lrow = lrow_b[:]
                mx = spool.tile([S, K], F32, tag="mx")
                idx = spool.tile([S, K], U32, tag="idx")
                nc.vector.max(out=mx[:], in_=lrow)
                nc.vector.max_index(out=idx[:], in_max=mx[:], in_values=lrow)
                negvmax = spool.tile([S, 1], F32, tag="nvm")
                nc.vector.tensor_scalar_mul(negvmax[:], mx[:, 0:1], -1.0)
                expt = epool.tile([S, L], F32, tag="expt")
                zsum = spool.tile([S, 1], F32, tag="zsum")
                nc.scalar.activation(
                    out=expt[:], in_=lrow, func=AF.Exp,
                    bias=negvmax[:, 0:1], accum_out=zsum[:, 0:1],
                )
                recipz = spool.tile([S, 1], F32, tag="rz")
                nc.vector.reciprocal(recipz[:], zsum[:])
                # w8 = exp(mx - vmax) * recipz   (paired with idx by rank)
                w8e = spool.tile([S, K], F32, tag="w8e")
                nc.vector.tensor_scalar_add(w8e[:], mx[:], negvmax[:, 0:1])
                nc.scalar.activation(out=w8e[:], in_=w8e[:], func=AF.Exp)
                w8 = spool.tile([S, K], F32, tag="w8")
                nc.vector.tensor_tensor(
                    out=w8[:], in0=w8e[:],
                    in1=recipz[:, 0:1].to_broadcast([S, K]), op=ALU.mult)
                # global row index = idx + b*4096 (as f32; exact for < 2^24)
                idxf = spool.tile([S, K], F32, tag="idxf")
                nc.vector.tensor_copy(out=idxf[:], in_=idx[:])
                nc.vector.tensor_scalar_add(idxf[:], idxf[:], float(b * L))
                # transpose idxf and w8 to [K, S] (rank on partitions)
                iw_ps = bps_pool.tile([K, 2 * S], F32, tag="bps")
                nc.tensor.transpose(iw_ps[:, 0:S], idxf[:], ident_sb[0:S, 0:S])
                nc.tensor.transpose(iw_ps[:, S:2 * S], w8[:], ident_sb[0:S, 0:S])
                iw_sb = spool.tile([K, 2 * S], F32, tag="iwsb")
                nc.vector.tensor_copy(out=iw_sb[:], in_=iw_ps[:])
                idxu = spool.tile([K, S], U32, tag="idxu")
                nc.vector.tensor_copy(out=idxu[:], in_=iw_sb[:, 0:S])

                # ---------------- PASS B: recompute top-8 rows ----------------
                for s in range(S):
                    q_sel = selpool.tile([K, D], F32, tag="qsel")
                    nc.gpsimd.indirect_dma_start(
                        out=q_sel[:], out_offset=None,
                        in_=qn_flat,
                        in_offset=bass.IndirectOffsetOnAxis(ap=idxu[:, s:s + 1], axis=0),
                    )
                    # q_selT chunks [128, 8] per kc
                    qst = selpool.tile([PD, KC * K], F32R, tag="qst")
                    for kc in range(KC):
                        t_ps = bps_pool.tile([PD, K], F32, tag="bps")
                        nc.tensor.transpose(
                            t_ps[:], q_sel[:, kc * PD:(kc + 1) * PD], ident_sb[0:K, 0:K])
                        nc.vector.tensor_copy(out=qst[:, kc * K:(kc + 1) * K], in_=t_ps[:])
                    # mm1 for selected rows: [8, 512]
                    hsel_ps = bps_pool.tile([K, D], F32, tag="bps2", bufs=1)
                    for kc in range(KC):
                        nc.tensor.matmul(
                            hsel_ps[:],
                            lhsT=qst[:, kc * K:(kc + 1) * K],
                            rhs=w1_sb[s][kc][:],
                            start=(kc == 0), stop=(kc == KC - 1),
                        )
                    hsel = selpool.tile([K, D], F32, tag="hsel")
                    nc.vector.tensor_tensor(
                        out=hsel[:], in0=hsel_ps[:],
                        in1=b1rep_sb[:, s * D:(s + 1) * D], op=ALU.add)
                    nc.vector.scalar_tensor_tensor(
                        out=hsel[:], in0=hsel[:], scalar=slope8_sb[:, s:s + 1],
                        in1=hsel[:], op0=ALU.mult, op1=ALU.max)
                    # transpose hsel -> [128, 8] chunks
                    hst = selpool.tile([PD, KC * K], F32R, tag="hst")
                    for kc in range(KC):
                        t_ps = bps_pool.tile([PD, K], F32, tag="bps")
                        nc.tensor.transpose(
                            t_ps[:], hsel[:, kc * PD:(kc + 1) * PD], ident_sb[0:K, 0:K])
                        nc.vector.tensor_copy(out=hst[:, kc * K:(kc + 1) * K], in_=t_ps[:])
                    xsel_ps = bps_pool.tile([K, D], F32, tag="bps2", bufs=1)
                    for kc in range(KC):
                        nc.tensor.matmul(
                            xsel_ps[:],
                            lhsT=hst[:, kc * K:(kc + 1) * K],
                            rhs=w2_sb[s][kc][:],
                            start=(kc == 0), stop=(kc == KC - 1),
                        )
                    xsel = selpool.tile([K, D], F32, tag="xsel")
                    nc.vector.tensor_tensor(
                        out=xsel[:], in0=xsel_ps[:],
                        in1=b2rep_sb[:, s * D:(s + 1) * D], op=ALU.add)
                    nc.scalar.activation(out=xsel[:], in_=xsel[:], func=AF.Tanh)
                    # combined[d] = sum_k w8[k] * xsel[k, d] -> [128, 1] per chunk
                    comb_ps = bps_pool.tile([PD, KC], F32, tag="bps")
                    for mc in range(KC):
                        nc.tensor.matmul(
                            comb_ps[:, mc:mc + 1],
                            lhsT=xsel[:, mc * PD:(mc + 1) * PD],
                            rhs=iw_sb[:, S + s:S + s + 1],
                            start=True, stop=True,
                        )
                    comb_sb = selpool.tile([PD, KC], F32, tag="combsb")
                    nc.vector.tensor_copy(out=comb_sb[:], in_=comb_ps[:])
                    nc.sync.dma_start(
                        out=out[s, :, b].rearrange("(mc p) -> p mc", p=PD),
                        in_=comb_sb[:],
                    )

            lrows = []
            # topk+passB for batch b is EMITTED inside batch b+1's tile loop
            # (after t==1) so its DVE/PE ops fill pipeline slack instead of
            # stalling the b->b+1 boundary; the last b runs after the loop.
            for b in range(B):
                lrow_b = lpool.tile([S, L], F32, tag="lrow", bufs=2)
                lrows.append(lrow_b)
                for t in range(T):
                    # load qT tile: [128, kc x 512] (d on partitions, rows free)
                    qt_tile = qtpool.tile([PD, KC * 512], F32R, tag="qt")
                    src = qt[b, :, t * 512:(t + 1) * 512].rearrange(
                        "(kc p) r -> p kc r", p=PD)
                    dst = qt_tile[:].rearrange("p (kc r) -> p kc r", r=512)
                    nc.sync.dma_start(out=dst, in_=src)

                    ht_tiles = []
                    # mm1 + leaky for both slots (interleaved for PE density)
                    for s in range(S):
                        ht = htpool.tile([PD, KC * 512], F32R, tag="ht")
                        ht_tiles.append(ht)
                        for mc in range(KC):
                            h_ps = hps_pool.tile([PD, 512], F32, tag="hps")
                            for kc in range(KC):
                                nc.tensor.matmul(
                                    h_ps[:],
                                    lhsT=w1_sb[s][kc][:, mc * PD:(mc + 1) * PD],
                                    rhs=qt_tile[:, kc * 512:(kc + 1) * 512],
                                    start=(kc == 0), stop=(kc == KC - 1),
                                )
                            # leaky: u = h + b1; out = max(slope*u, u).
                            # The bias add runs on ACT (Identity+bias from an
                            # ACT-local tile) to offload DVE; the DVE mult-max
                            # then carries a single ACT wait (TensorScalarPtr
                            # allows only one sync wait).
                            col = s * KC + mc
                            v = tmppool.tile([PD, 512], F32, tag="v")
                            nc.scalar.activation(
                                out=v[:], in_=h_ps[:], func=AF.Identity,
                                bias=b1tA[:, col:col + 1],
                            )
                            nc.vector.scalar_tensor_tensor(
                                out=ht[:, mc * 512:(mc + 1) * 512],
                                in0=v[:],
                                scalar=slopet_sb[:, s:s + 1],
                                in1=v[:],
                                op0=ALU.mult, op1=ALU.max,
                            )
                    xt_tiles = []
                    for s in range(S):
                        ht = ht_tiles[s]
                        xt = xtpool.tile([PD, KC * 512], F32R, tag="xt")
                        xt_tiles.append(xt)
                        for mc in range(KC):
                            x_ps = xps_pool.tile([PD, 512], F32, tag="xps")
                            for kc in range(KC):
                                nc.tensor.matmul(
                                    x_ps[:],
                                    lhsT=w2_sb[s][kc][:, mc * PD:(mc + 1) * PD],
                                    rhs=ht[:, kc * 512:(kc + 1) * 512],
                                    start=(kc == 0), stop=(kc == KC - 1),
                                )
                            col = s * KC + mc
                            nc.scalar.activation(
                                out=xt[:, mc * 512:(mc + 1) * 512], in_=x_ps[:],
                                func=AF.Tanh, bias=b2t_sb[:, col:col + 1],
                            )
                    # gate matvec per slot ([1, 512] psum each; PE out must
                    # start at partition 0). Engine ops can only address SBUF
                    # partitions 0/32/64/96, so stage the row at partition 0
                    # (with +bg) and DMA it into lrow_b's partition s.
                    for s in range(S):
                        g_ps = gps_pool.tile([1, 512], F32, tag="gps")
                        for kc in range(KC):
                            nc.tensor.matmul(
                                g_ps[:],
                                lhsT=wgt_sb[:, s * KC + kc:s * KC + kc + 1],
                                rhs=xt_tiles[s][:, kc * 512:(kc + 1) * 512],
                                start=(kc == 0), stop=(kc == KC - 1),
                            )
                        lstage = tmppool.tile([1, 512], F32, tag="lstage")
                        nc.scalar.activation(
                            out=lstage[:], in_=g_ps[:], func=AF.Identity,
                            bias=bgtA[0:1, s:s + 1],
                        )
                        nc.sync.dma_start(
                            out=lrow_b[s:s + 1, t * 512:(t + 1) * 512],
                            in_=lstage[:])

                    if t == 1 and b > 0:
                        topk_passB(b - 1, lrows[b - 1])


            topk_passB(B - 1, lrows[B - 1])

    nc.compile()  # Bacc passes: reg alloc, DCE, wait splitting (TRN2 1-wait rule)
    _PROGRAM_CACHE["nc"] = nc
    return nc


def _prep_in_maps(query, W1, b1, W2, b2, Wg, bg):
    query = np.ascontiguousarray(query, dtype=np.float32)
    qt = np.ascontiguousarray(query.transpose(0, 2, 1))
    slopes = (0.01 + 0.2 / N_MEM * np.arange(N_MEM, dtype=np.float32))
    ident = np.eye(PD, dtype=np.float32)
    in_maps = []
    for c in range(NCORES):
        sl = slice(S * c, S * (c + 1))
        b1c = np.asarray(b1[sl], np.float32)     # [S, D]
        b2c = np.asarray(b2[sl], np.float32)
        wgc = np.asarray(Wg[sl], np.float32)
        slc = slopes[sl]                          # [S]
        sb1c = slc[:, None] * b1c
        def t128(a):  # [S, D] -> [128, S*KC]
            return np.ascontiguousarray(
                a.reshape(S, KC, PD).transpose(2, 0, 1).reshape(PD, S * KC))
        misc128 = np.concatenate([
            t128(b1c), t128(sb1c), t128(b2c),
            np.broadcast_to(slc[None, :], (PD, S)),
        ], axis=1).astype(np.float32)
        misc8 = np.concatenate([
            np.broadcast_to(b1c.reshape(1, S * D), (K, S * D)),
            np.broadcast_to(b2c.reshape(1, S * D), (K, S * D)),
            np.broadcast_to(slc[None, :], (K, S)),
        ], axis=1).astype(np.float32)
        in_maps.append({
            "qt": qt,
            "qn": query,
            "w1": np.ascontiguousarray(W1[sl], np.float32),
            "w2": np.ascontiguousarray(W2[sl], np.float32),
            "misc128": np.ascontiguousarray(misc128),
            "wgt": t128(wgc),
            "bgt": np.asarray(bg[sl], np.float32).reshape(1, S),
            "misc8": np.ascontiguousarray(misc8),
            "ident": ident,
        })
    return in_maps


def kernel(query, W1, b1, W2, b2, Wg, bg, topk, **_ignored):
    assert int(topk) == K, f"kernel hardcodes topk={K}, got {topk}"
    nc = _build_program()
    in_maps = _prep_in_maps(
        np.asarray(query), np.asarray(W1), np.asarray(b1), np.asarray(W2),
        np.asarray(b2), np.asarray(Wg), np.asarray(bg))
    res = bass_utils.run_bass_kernel_spmd(nc, in_maps, core_ids=list(range(NCORES)))
    outs = res.results
    # outs[c]["out"] is [S, D, B]; assemble [B, N_MEM, D]
    m = np.empty((B, N_MEM, D), dtype=np.float32)
    for c in range(NCORES):
        oc = np.asarray(outs[c]["out"])  # [S, D, B]
        for s in range(S):
            m[:, S * c + s, :] = oc[s].T
    norm = np.maximum(np.linalg.norm(m.astype(np.float64), axis=-1, keepdims=True),
                      1e-12).astype(np.float32)
    return (m / norm).astype(np.float32)


# revision 22
# speedup vs baseline: 1.0478x; 1.0249x over previous
"""Trainium2 Bass kernel for the topk_masking memory-module problem.

Computation (reference semantics):
  For each of n=16 memory slots l:
    h = LeakyReLU_{slope_l}(q @ W1[l] + b1[l])          # [b, L, d]
    x = tanh(h @ W2[l] + b2[l])                          # [b, L, d]
    logits = x @ Wg[l] + bg[l]                           # [b, L]
    w = softmax(logits over L); top8 (values+indices)
    combined[b] = sum_k w_topk[k] * x[b, idx_k]          # [b, d]
  out[b, l, :] = normalize(combined over d)

Sharding: expert-parallel over the 16 memory slots -> 2 slots per core on
8 cores.  Each core runs the full [4, 4096, 512] query through its two
slots.  Device does everything except the final L2 normalize (host, cheap).

Device algorithm per core (pass A computes logits while discarding x; the
top-8 rows of x are recomputed in pass B from the gathered q rows):
  pass A: for b, for t (8 row-tiles of 512):
    hT = leaky(W1^T-chunks @ qT-tile + b1)   (transposed pipeline, d on
    xT = tanh(W2-chunks @ hT + b2)            partitions, rows on free)
    logits[2b+s, t*512:] = Wg . xT + bg      (PE matvec)
  per b: max8 + max_index give top-8 values+indices (paired, descending);
    softmax stats via Exp activation with accumulate; weights from values.
  pass B per (b, s): indirect-DMA gather the 8 q rows, recompute their x
    (tiny matmuls), then combined = x_sel^T @ w8 on PE; DMA to out[s,:,b].
"""

import numpy as np

import concourse.bass as bass
import concourse.bacc as bacc
import concourse.mybir as mybir
from concourse import bass_utils
from concourse.tile import TileContext

F32 = mybir.dt.float32
F32R = mybir.dt.float32r
U32 = mybir.dt.uint32
AF = mybir.ActivationFunctionType
ALU = mybir.AluOpType

B = 4
L = 4096
D = 512
N_MEM = 16
NCORES = 8
S = N_MEM // NCORES  # 2 slots per core
K = 8
T = L // 512  # 8 row-tiles per batch
PD = 128     # partition dim
KC = D // PD  # 4 contraction chunks

# The heavy pipeline runs the PE in float32r (fp32 operands, 1 cycle/row vs
# 4 for plain float32; slightly reduced multiply precision).  Top-k selection
# is sensitive to logit error, so this choice is validated against the
# reference in test.py on both CPU- and axon-generated datasets.
_PROGRAM_CACHE = {}


def _build_program():
    if "nc" in _PROGRAM_CACHE:
        return _PROGRAM_CACHE["nc"]

    nc = bacc.Bacc("TRN2", debug=False, enable_asserts=False, num_devices=NCORES)

    qt = nc.dram_tensor("qt", [B, D, L], F32R, kind="ExternalInput").ap()
    qn = nc.dram_tensor("qn", [B, L, D], F32, kind="ExternalInput").ap()
    w1 = nc.dram_tensor("w1", [S, D, D], F32R, kind="ExternalInput").ap()
    w2 = nc.dram_tensor("w2", [S, D, D], F32R, kind="ExternalInput").ap()
    # small constants packed into two tensors (one DMA each) so consumers
    # carry few semaphore waits: misc128 = [b1t | sb1t | b2t | slopet],
    # misc8 = [b1rep | b2rep | slope8]
    misc128 = nc.dram_tensor("misc128", [PD, 3 * S * KC + S], F32,
                             kind="ExternalInput").ap()
    wgt = nc.dram_tensor("wgt", [PD, S * KC], F32R, kind="ExternalInput").ap()
    bgt = nc.dram_tensor("bgt", [1, S], F32, kind="ExternalInput").ap()
    misc8 = nc.dram_tensor("misc8", [K, 2 * S * D + S], F32,
                           kind="ExternalInput").ap()
    ident = nc.dram_tensor("ident", [PD, PD], F32, kind="ExternalInput").ap()
    out = nc.dram_tensor("out", [S, D, B], F32, kind="ExternalOutput").ap()

    qn_flat = qn.rearrange("b l d -> (b l) d")

    with TileContext(nc) as tc:
        with (
            tc.tile_pool(name="consts", bufs=1) as cpool,
            tc.tile_pool(name="weights", bufs=1) as wpool,
            tc.tile_pool(name="qtp", bufs=2) as qtpool,
            tc.tile_pool(name="ht", bufs=3) as htpool,
            tc.tile_pool(name="xt", bufs=3) as xtpool,
            tc.tile_pool(name="tmp", bufs=3) as tmppool,
            tc.tile_pool(name="logits", bufs=1) as lpool,
            tc.tile_pool(name="small", bufs=8) as spool,
            tc.tile_pool(name="expp", bufs=1) as epool,
            tc.tile_pool(name="selp", bufs=2) as selpool,
            tc.tile_pool(name="hps", bufs=2, space="PSUM") as hps_pool,
            tc.tile_pool(name="xps", bufs=2, space="PSUM") as xps_pool,
            tc.tile_pool(name="gps", bufs=1, space="PSUM") as gps_pool,
            tc.tile_pool(name="bps", bufs=2, space="PSUM") as bps_pool,
        ):
            # --- weights for mm1 slot 0 first: the PE's first matmul only
            # needs w1[0][0] + the first qt tile (sync queue), so their DMAs
            # lead both queues ---
            w1_sb = [[wpool.tile([PD, D], F32R, name=f"w1sb_{s}_{kc}", tag=f"w1_{s}_{kc}")
                      for kc in range(KC)] for s in range(S)]
            w2_sb = [[wpool.tile([PD, D], F32R, name=f"w2sb_{s}_{kc}", tag=f"w2_{s}_{kc}")
                      for kc in range(KC)] for s in range(S)]
            for kc in range(KC):
                nc.gpsimd.dma_start(out=w1_sb[0][kc][:], in_=w1[0, kc * PD:(kc + 1) * PD, :])

            # --- persistent constants / weights in SBUF ---
            misc128_sb = cpool.tile_from(misc128, forced_dma_engine=mybir.EngineType.Pool)
            wgt_sb = cpool.tile_from(wgt, forced_dma_engine=mybir.EngineType.Pool)
            bgt_sb = cpool.tile_from(bgt, forced_dma_engine=mybir.EngineType.Pool)
            misc8_sb = cpool.tile_from(misc8, forced_dma_engine=mybir.EngineType.Pool)
            ident_sb = cpool.tile_from(ident, forced_dma_engine=mybir.EngineType.Pool)
            # TensorScalarPtr (scalar-operand-from-AP) instructions can carry
            # only one sync wait, so scalar sources must be same-engine local:
            # stage DVE-consumed constants through a DVE copy and ACT-consumed
            # biases through an ACT copy.  After these copies each engine has
            # observed the const DMA sem once, so no later op re-waits on it.
            misc128L = cpool.tile([PD, 3 * S * KC + S], F32, name="misc128L")
            nc.vector.tensor_copy(out=misc128L[:], in_=misc128_sb[:])
            misc8L = cpool.tile([K, 2 * S * D + S], F32, name="misc8L")
            nc.vector.tensor_copy(out=misc8L[:], in_=misc8_sb[:])
            m128A = cpool.tile([PD, 3 * S * KC], F32, name="m128A")
            nc.scalar.copy(out=m128A[:], in_=misc128_sb[:, 0:3 * S * KC])
            b1tA = m128A[:, 0:S * KC]
            b2tA = m128A[:, 2 * S * KC:3 * S * KC]
            bgtA = cpool.tile([1, S], F32, name="bgtA")
            nc.scalar.copy(out=bgtA[:], in_=bgt_sb[:])
            b1t_sb = misc128L[:, 0:S * KC]
            sb1t_sb = misc128L[:, S * KC:2 * S * KC]
            b2t_sb = b2tA[:]
            slopet_sb = misc128L[:, 3 * S * KC:3 * S * KC + S]
            b1rep_sb = misc8L[:, 0:S * D]
            b2rep_sb = misc8L[:, S * D:2 * S * D]
            slope8_sb = misc8L[:, 2 * S * D:2 * S * D + S]

            # remaining weights (w1 slot 1, then w2) load behind the consts
            for kc in range(KC):
                nc.gpsimd.dma_start(out=w1_sb[1][kc][:], in_=w1[1, kc * PD:(kc + 1) * PD, :])
            for s in range(S):
                for kc in range(KC):
                    nc.gpsimd.dma_start(out=w2_sb[s][kc][:], in_=w2[s, kc * PD:(kc + 1) * PD, :])

            # ---------------- PASS A + per-b topk / pass B ----------------
            def topk_passB(b, lrow_b, cand):
                # ---- final top-k merge over the 64 per-tile candidates ----
                cand_v, cand_p = cand
                lrow = lrow_b[:]
                mx = spool.tile([S, K], F32, tag="mx")
                nc.vector.max(out=mx[:], in_=cand_v[:])
                masked = spool.tile([S, T * K], F32, tag="msk")
                nc.vector.match_replace(out=masked[:], in_to_replace=mx[:],
                                        in_values=cand_v[:], imm_value=-1e30)
                gtm = spool.tile([S, T * K], F32, tag="gtm")
                nc.vector.tensor_tensor(out=gtm[:], in0=cand_v[:], in1=masked[:],
                                        op=ALU.is_gt)
                mpk = spool.tile([S, T * K], F32, tag="mpk")
                nc.vector.tensor_tensor(out=mpk[:], in0=gtm[:], in1=cand_p[:],
                                        op=ALU.mult)
                # pk8: the 8 selected packs, descending by row index
                pk8 = spool.tile([S, K], F32, tag="pk8")
                nc.vector.max(out=pk8[:], in_=mpk[:])
                t3 = spool.tile([S, K], F32, tag="t3")
                nc.vector.tensor_scalar_mul(t3[:], pk8[:], 0.125)
                iu = spool.tile([S, K], U32, tag="iu")
                nc.vector.tensor_copy(out=iu[:], in_=t3[:])      # -> local idx+1
                tif2 = spool.tile([S, K], F32, tag="tif2")
                nc.vector.tensor_copy(out=tif2[:], in_=iu[:])
                # lhat + offset(-2) folded below; t5 = pk8 - 8*(idx+1)
                t5 = spool.tile([S, K], F32, tag="t5")
                nc.vector.scalar_tensor_tensor(
                    out=t5[:], in0=tif2[:], scalar=-8.0, in1=pk8[:],
                    op0=ALU.mult, op1=ALU.add)
                negvmax = spool.tile([S, 1], F32, tag="nvm")
                nc.vector.tensor_scalar_mul(negvmax[:], mx[:, 0:1], -1.0)
                expt = epool.tile([S, L], F32, tag="expt")
                zsum = spool.tile([S, 1], F32, tag="zsum")
                nc.scalar.activation(
                    out=expt[:], in_=lrow, func=AF.Exp,
                    bias=negvmax[:, 0:1], accum_out=zsum[:, 0:1],
                )
                recipz = spool.tile([S, 1], F32, tag="rz")
                nc.vector.reciprocal(recipz[:], zsum[:])
                # w8 = exp(lhat - vmax) * recipz, index-desc order (pass B
                # gathers rows in the same order, so any consistent order works)
                w8e = spool.tile([S, K], F32, tag="w8e")
                nc.vector.tensor_scalar(
                    out=w8e[:], in0=t5[:], scalar1=negvmax[:, 0:1],
                    scalar2=-2.0, op0=ALU.add, op1=ALU.add)
                nc.scalar.activation(out=w8e[:], in_=w8e[:], func=AF.Exp)
                w8 = spool.tile([S, K], F32, tag="w8")
                nc.vector.tensor_tensor(
                    out=w8[:], in0=w8e[:],
                    in1=recipz[:, 0:1].to_broadcast([S, K]), op=ALU.mult)
                # global row index = (local idx+1) - 1 + b*4096
                idxf = spool.tile([S, K], F32, tag="idxf")
                nc.vector.tensor_scalar_add(idxf[:], tif2[:], float(b * L - 1))
                # transpose idxf and w8 to [K, S] (rank on partitions)
                iw_ps = bps_pool.tile([K, 2 * S], F32, tag="bps")
                nc.tensor.transpose(iw_ps[:, 0:S], idxf[:], ident_sb[0:S, 0:S])
                nc.tensor.transpose(iw_ps[:, S:2 * S], w8[:], ident_sb[0:S, 0:S])
                iw_sb = spool.tile([K, 2 * S], F32, tag="iwsb")
                nc.vector.tensor_copy(out=iw_sb[:], in_=iw_ps[:])
                idxu = spool.tile([K, S], U32, tag="idxu")
                nc.vector.tensor_copy(out=idxu[:], in_=iw_sb[:, 0:S])

                # ---------------- PASS B: recompute top-8 rows ----------------
                for s in range(S):
                    q_sel = selpool.tile([K, D], F32, tag="qsel")
                    nc.gpsimd.indirect_dma_start(
                        out=q_sel[:], out_offset=None,
                        in_=qn_flat,
                        in_offset=bass.IndirectOffsetOnAxis(ap=idxu[:, s:s + 1], axis=0),
                    )
                    # q_selT chunks [128, 8] per kc
                    qst = selpool.tile([PD, KC * K], F32R, tag="qst")
                    for kc in range(KC):
                        t_ps = bps_pool.tile([PD, K], F32, tag="bps")
                        nc.tensor.transpose(
                            t_ps[:], q_sel[:, kc * PD:(kc + 1) * PD], ident_sb[0:K, 0:K])
                        nc.vector.tensor_copy(out=qst[:, kc * K:(kc + 1) * K], in_=t_ps[:])
                    # mm1 for selected rows: [8, 512]
                    hsel_ps = bps_pool.tile([K, D], F32, tag="bps2", bufs=1)
                    for kc in range(KC):
                        nc.tensor.matmul(
                            hsel_ps[:],
                            lhsT=qst[:, kc * K:(kc + 1) * K],
                            rhs=w1_sb[s][kc][:],
                            start=(kc == 0), stop=(kc == KC - 1),
                        )
                    hsel = selpool.tile([K, D], F32, tag="hsel")
                    nc.vector.tensor_tensor(
                        out=hsel[:], in0=hsel_ps[:],
                        in1=b1rep_sb[:, s * D:(s + 1) * D], op=ALU.add)
                    nc.vector.scalar_tensor_tensor(
                        out=hsel[:], in0=hsel[:], scalar=slope8_sb[:, s:s + 1],
                        in1=hsel[:], op0=ALU.mult, op1=ALU.max)
                    # transpose hsel -> [128, 8] chunks
                    hst = selpool.tile([PD, KC * K], F32R, tag="hst")
                    for kc in range(KC):
                        t_ps = bps_pool.tile([PD, K], F32, tag="bps")
                        nc.tensor.transpose(
                            t_ps[:], hsel[:, kc * PD:(kc + 1) * PD], ident_sb[0:K, 0:K])
                        nc.vector.tensor_copy(out=hst[:, kc * K:(kc + 1) * K], in_=t_ps[:])
                    xsel_ps = bps_pool.tile([K, D], F32, tag="bps2", bufs=1)
                    for kc in range(KC):
                        nc.tensor.matmul(
                            xsel_ps[:],
                            lhsT=hst[:, kc * K:(kc + 1) * K],
                            rhs=w2_sb[s][kc][:],
                            start=(kc == 0), stop=(kc == KC - 1),
                        )
                    xsel = selpool.tile([K, D], F32, tag="xsel")
                    nc.vector.tensor_tensor(
                        out=xsel[:], in0=xsel_ps[:],
                        in1=b2rep_sb[:, s * D:(s + 1) * D], op=ALU.add)
                    nc.scalar.activation(out=xsel[:], in_=xsel[:], func=AF.Tanh)
                    # combined[d] = sum_k w8[k] * xsel[k, d] -> [128, 1] per chunk
                    comb_ps = bps_pool.tile([PD, KC], F32, tag="bps")
                    for mc in range(KC):
                        nc.tensor.matmul(
                            comb_ps[:, mc:mc + 1],
                            lhsT=xsel[:, mc * PD:(mc + 1) * PD],
                            rhs=iw_sb[:, S + s:S + s + 1],
                            start=True, stop=True,
                        )
                    comb_sb = selpool.tile([PD, KC], F32, tag="combsb")
                    nc.vector.tensor_copy(out=comb_sb[:], in_=comb_ps[:])
                    nc.sync.dma_start(
                        out=out[s, :, b].rearrange("(mc p) -> p mc", p=PD),
                        in_=comb_sb[:],
                    )

            lrows = []
            cands = []
            # topk+passB for batch b is EMITTED inside batch b+1's tile loop
            # (after t==1) so its DVE/PE ops fill pipeline slack instead of
            # stalling the b->b+1 boundary; the last b runs after the loop.
            for b in range(B):
                lrow_b = lpool.tile([S, L], F32, tag="lrow", bufs=2)
                lrows.append(lrow_b)
                cand_v = lpool.tile([S, T * K], F32, tag="candv", bufs=2)
                cand_p = lpool.tile([S, T * K], F32, tag="candp", bufs=2)
                cands.append((cand_v, cand_p))
                for t in range(T):
                    # load qT tile: [128, kc x 512] (d on partitions, rows free)
                    qt_tile = qtpool.tile([PD, KC * 512], F32R, tag="qt")
                    src = qt[b, :, t * 512:(t + 1) * 512].rearrange(
                        "(kc p) r -> p kc r", p=PD)
                    dst = qt_tile[:].rearrange("p (kc r) -> p kc r", r=512)
                    nc.sync.dma_start(out=dst, in_=src)

                    ht_tiles = []
                    # mm1 + leaky for both slots (interleaved for PE density)
                    for s in range(S):
                        ht = htpool.tile([PD, KC * 512], F32R, tag="ht")
                        ht_tiles.append(ht)
                        for mc in range(KC):
                            h_ps = hps_pool.tile([PD, 512], F32, tag="hps")
                            for kc in range(KC):
                                nc.tensor.matmul(
                                    h_ps[:],
                                    lhsT=w1_sb[s][kc][:, mc * PD:(mc + 1) * PD],
                                    rhs=qt_tile[:, kc * 512:(kc + 1) * 512],
                                    start=(kc == 0), stop=(kc == KC - 1),
                                )
                            # leaky: u = h + b1; out = max(slope*u, u).
                            # The bias add runs on ACT (Identity+bias from an
                            # ACT-local tile) to offload DVE; the DVE mult-max
                            # then carries a single ACT wait (TensorScalarPtr
                            # allows only one sync wait).
                            col = s * KC + mc
                            v = tmppool.tile([PD, 512], F32, tag="v")
                            nc.scalar.activation(
                                out=v[:], in_=h_ps[:], func=AF.Identity,
                                bias=b1tA[:, col:col + 1],
                            )
                            nc.vector.scalar_tensor_tensor(
                                out=ht[:, mc * 512:(mc + 1) * 512],
                                in0=v[:],
                                scalar=slopet_sb[:, s:s + 1],
                                in1=v[:],
                                op0=ALU.mult, op1=ALU.max,
                            )
                    xt_tiles = []
                    for s in range(S):
                        ht = ht_tiles[s]
                        xt = xtpool.tile([PD, KC * 512], F32R, tag="xt")
                        xt_tiles.append(xt)
                        for mc in range(KC):
                            x_ps = xps_pool.tile([PD, 512], F32, tag="xps")
                            for kc in range(KC):
                                nc.tensor.matmul(
                                    x_ps[:],
                                    lhsT=w2_sb[s][kc][:, mc * PD:(mc + 1) * PD],
                                    rhs=ht[:, kc * 512:(kc + 1) * 512],
                                    start=(kc == 0), stop=(kc == KC - 1),
                                )
                            col = s * KC + mc
                            nc.scalar.activation(
                                out=xt[:, mc * 512:(mc + 1) * 512], in_=x_ps[:],
                                func=AF.Tanh, bias=b2t_sb[:, col:col + 1],
                            )
                    # gate matvec per slot ([1, 512] psum each; PE out must
                    # start at partition 0). Engine ops can only address SBUF
                    # partitions 0/32/64/96, so stage the row at partition 0
                    # (with +bg) and DMA it into lrow_b's partition s.
                    for s in range(S):
                        g_ps = gps_pool.tile([1, 512], F32, tag="gps")
                        for kc in range(KC):
                            nc.tensor.matmul(
                                g_ps[:],
                                lhsT=wgt_sb[:, s * KC + kc:s * KC + kc + 1],
                                rhs=xt_tiles[s][:, kc * 512:(kc + 1) * 512],
                                start=(kc == 0), stop=(kc == KC - 1),
                            )
                        lstage = tmppool.tile([1, 512], F32, tag="lstage")
                        nc.scalar.activation(
                            out=lstage[:], in_=g_ps[:], func=AF.Identity,
                            bias=bgtA[0:1, s:s + 1],
                        )
                        nc.sync.dma_start(
                            out=lrow_b[s:s + 1, t * 512:(t + 1) * 512],
                            in_=lstage[:])

                    # hierarchical top-k, overlapped stage: per-tile top-8
                    # values + packed (index, clamped value) candidates.
                    # pack = 8*(local_idx+1) + (clamp(l, +-1.9) + 2): integer
                    # part recovers the index under truncation OR
                    # round-to-nearest; the fraction carries the logit to
                    # ~4e-3, used only for softmax weights (selection uses
                    # exact values in cand_v).
                    lslice = lrow_b[:, t * 512:(t + 1) * 512]
                    nc.vector.max(out=cand_v[:, t * K:(t + 1) * K], in_=lslice)
                    ti = spool.tile([S, K], U32, tag="ti")
                    nc.vector.max_index(out=ti[:], in_max=cand_v[:, t * K:(t + 1) * K],
                                        in_values=lslice)
                    tif = spool.tile([S, K], F32, tag="tif")
                    nc.vector.tensor_copy(out=tif[:], in_=ti[:])
                    t1 = spool.tile([S, K], F32, tag="t1")
                    nc.vector.tensor_scalar(
                        out=t1[:], in0=tif[:], scalar1=float(t * 512 + 1),
                        scalar2=8.0, op0=ALU.add, op1=ALU.mult)
                    t2 = spool.tile([S, K], F32, tag="t2")
                    nc.vector.tensor_scalar(
                        out=t2[:], in0=cand_v[:, t * K:(t + 1) * K],
                        scalar1=1.9, scalar2=-1.9, op0=ALU.min, op1=ALU.max)
                    nc.vector.scalar_tensor_tensor(
                        out=cand_p[:, t * K:(t + 1) * K], in0=t2[:], scalar=2.0,
                        in1=t1[:], op0=ALU.add, op1=ALU.add)

                    if t == 1 and b > 0:
                        topk_passB(b - 1, lrows[b - 1], cands[b - 1])


            topk_passB(B - 1, lrows[B - 1], cands[B - 1])

    nc.compile()  # Bacc passes: reg alloc, DCE, wait splitting (TRN2 1-wait rule)
    _PROGRAM_CACHE["nc"] = nc
    return nc


def _prep_in_maps(query, W1, b1, W2, b2, Wg, bg):
    query = np.ascontiguousarray(query, dtype=np.float32)
    qt = np.ascontiguousarray(query.transpose(0, 2, 1))
    slopes = (0.01 + 0.2 / N_MEM * np.arange(N_MEM, dtype=np.float32))
    ident = np.eye(PD, dtype=np.float32)
    in_maps = []
    for c in range(NCORES):
        sl = slice(S * c, S * (c + 1))
        b1c = np.asarray(b1[sl], np.float32)     # [S, D]
        b2c = np.asarray(b2[sl], np.float32)
        wgc = np.asarray(Wg[sl], np.float32)
        slc = slopes[sl]                          # [S]
        sb1c = slc[:, None] * b1c
        def t128(a):  # [S, D] -> [128, S*KC]
            return np.ascontiguousarray(
                a.reshape(S, KC, PD).transpose(2, 0, 1).reshape(PD, S * KC))
        misc128 = np.concatenate([
            t128(b1c), t128(sb1c), t128(b2c),
            np.broadcast_to(slc[None, :], (PD, S)),
        ], axis=1).astype(np.float32)
        misc8 = np.concatenate([
            np.broadcast_to(b1c.reshape(1, S * D), (K, S * D)),
            np.broadcast_to(b2c.reshape(1, S * D), (K, S * D)),
            np.broadcast_to(slc[None, :], (K, S)),
        ], axis=1).astype(np.float32)
        in_maps.append({
            "qt": qt,
            "qn": query,
            "w1": np.ascontiguousarray(W1[sl], np.float32),
            "w2": np.ascontiguousarray(W2[sl], np.float32),
            "misc128": np.ascontiguousarray(misc128),
            "wgt": t128(wgc),
            "bgt": np.asarray(bg[sl], np.float32).reshape(1, S),
            "misc8": np.ascontiguousarray(misc8),
            "ident": ident,
        })
    return in_maps


def kernel(query, W1, b1, W2, b2, Wg, bg, topk, **_ignored):
    assert int(topk) == K, f"kernel hardcodes topk={K}, got {topk}"
    nc = _build_program()
    in_maps = _prep_in_maps(
        np.asarray(query), np.asarray(W1), np.asarray(b1), np.asarray(W2),
        np.asarray(b2), np.asarray(Wg), np.asarray(bg))
    res = bass_utils.run_bass_kernel_spmd(nc, in_maps, core_ids=list(range(NCORES)))
    outs = res.results
    # outs[c]["out"] is [S, D, B]; assemble [B, N_MEM, D]
    m = np.empty((B, N_MEM, D), dtype=np.float32)
    for c in range(NCORES):
        oc = np.asarray(outs[c]["out"])  # [S, D, B]
        for s in range(S):
            m[:, S * c + s, :] = oc[s].T
    norm = np.maximum(np.linalg.norm(m.astype(np.float64), axis=-1, keepdims=True),
                      1e-12).astype(np.float32)
    return (m / norm).astype(np.float32)


# revision 23
# speedup vs baseline: 1.0705x; 1.0216x over previous
"""Trainium2 Bass kernel for the topk_masking memory-module problem.

Computation (reference semantics):
  For each of n=16 memory slots l:
    h = LeakyReLU_{slope_l}(q @ W1[l] + b1[l])          # [b, L, d]
    x = tanh(h @ W2[l] + b2[l])                          # [b, L, d]
    logits = x @ Wg[l] + bg[l]                           # [b, L]
    w = softmax(logits over L); top8 (values+indices)
    combined[b] = sum_k w_topk[k] * x[b, idx_k]          # [b, d]
  out[b, l, :] = normalize(combined over d)

Sharding: expert-parallel over the 16 memory slots -> 2 slots per core on
8 cores.  Each core runs the full [4, 4096, 512] query through its two
slots.  Device does everything except the final L2 normalize (host, cheap).

Device algorithm per core (pass A computes logits while discarding x; the
top-8 rows of x are recomputed in pass B from the gathered q rows):
  pass A: for b, for t (8 row-tiles of 512):
    hT = leaky(W1^T-chunks @ qT-tile + b1)   (transposed pipeline, d on
    xT = tanh(W2-chunks @ hT + b2)            partitions, rows on free)
    logits[2b+s, t*512:] = Wg . xT + bg      (PE matvec)
  per b: max8 + max_index give top-8 values+indices (paired, descending);
    softmax stats via Exp activation with accumulate; weights from values.
  pass B per (b, s): indirect-DMA gather the 8 q rows, recompute their x
    (tiny matmuls), then combined = x_sel^T @ w8 on PE; DMA to out[s,:,b].
"""

import numpy as np

import concourse.bass as bass
import concourse.bacc as bacc
import concourse.mybir as mybir
from concourse import bass_utils
from concourse.tile import TileContext

F32 = mybir.dt.float32
F32R = mybir.dt.float32r
U32 = mybir.dt.uint32
AF = mybir.ActivationFunctionType
ALU = mybir.AluOpType

B = 4
L = 4096
D = 512
N_MEM = 16
NCORES = 8
S = N_MEM // NCORES  # 2 slots per core
K = 8
T = L // 512  # 8 row-tiles per batch
PD = 128     # partition dim
KC = D // PD  # 4 contraction chunks

# The heavy pipeline runs the PE in float32r (fp32 operands, 1 cycle/row vs
# 4 for plain float32; slightly reduced multiply precision).  Top-k selection
# is sensitive to logit error, so this choice is validated against the
# reference in test.py on both CPU- and axon-generated datasets.
_PROGRAM_CACHE = {}


def _build_program():
    if "nc" in _PROGRAM_CACHE:
        return _PROGRAM_CACHE["nc"]

    nc = bacc.Bacc("TRN2", debug=False, enable_asserts=False, num_devices=NCORES)

    qt = nc.dram_tensor("qt", [B, D, L], F32R, kind="ExternalInput").ap()
    qn = nc.dram_tensor("qn", [B, L, D], F32, kind="ExternalInput").ap()
    w1 = nc.dram_tensor("w1", [S, D, D], F32R, kind="ExternalInput").ap()
    w2 = nc.dram_tensor("w2", [S, D, D], F32R, kind="ExternalInput").ap()
    # small constants packed into two tensors (one DMA each) so consumers
    # carry few semaphore waits: misc128 = [b1t | sb1t | b2t | slopet],
    # misc8 = [b1rep | b2rep | slope8]
    misc128 = nc.dram_tensor("misc128", [PD, 3 * S * KC + S], F32,
                             kind="ExternalInput").ap()
    wgt = nc.dram_tensor("wgt", [PD, S * KC], F32R, kind="ExternalInput").ap()
    bgt = nc.dram_tensor("bgt", [1, S], F32, kind="ExternalInput").ap()
    misc8 = nc.dram_tensor("misc8", [K, 2 * S * D + S], F32,
                           kind="ExternalInput").ap()
    ident = nc.dram_tensor("ident", [PD, PD], F32, kind="ExternalInput").ap()
    out = nc.dram_tensor("out", [S, D, B], F32, kind="ExternalOutput").ap()

    qn_flat = qn.rearrange("b l d -> (b l) d")

    with TileContext(nc) as tc:
        with (
            tc.tile_pool(name="consts", bufs=1) as cpool,
            tc.tile_pool(name="weights", bufs=1) as wpool,
            tc.tile_pool(name="qtp", bufs=2) as qtpool,
            tc.tile_pool(name="ht", bufs=3) as htpool,
            tc.tile_pool(name="xt", bufs=3) as xtpool,
            tc.tile_pool(name="tmp", bufs=3) as tmppool,
            tc.tile_pool(name="logits", bufs=1) as lpool,
            tc.tile_pool(name="small", bufs=8) as spool,
            tc.tile_pool(name="expp", bufs=1) as epool,
            tc.tile_pool(name="selp", bufs=2) as selpool,
            tc.tile_pool(name="hps", bufs=3, space="PSUM") as hps_pool,
            tc.tile_pool(name="xps", bufs=2, space="PSUM") as xps_pool,
            tc.tile_pool(name="gps", bufs=1, space="PSUM") as gps_pool,
            tc.tile_pool(name="bps", bufs=1, space="PSUM") as bps_pool,
        ):
            # --- weights for mm1 slot 0 first: the PE's first matmul only
            # needs w1[0][0] + the first qt tile (sync queue), so their DMAs
            # lead both queues ---
            w1_sb = [[wpool.tile([PD, D], F32R, name=f"w1sb_{s}_{kc}", tag=f"w1_{s}_{kc}")
                      for kc in range(KC)] for s in range(S)]
            w2_sb = [[wpool.tile([PD, D], F32R, name=f"w2sb_{s}_{kc}", tag=f"w2_{s}_{kc}")
                      for kc in range(KC)] for s in range(S)]
            for kc in range(KC):
                nc.gpsimd.dma_start(out=w1_sb[0][kc][:], in_=w1[0, kc * PD:(kc + 1) * PD, :])

            # --- persistent constants / weights in SBUF ---
            misc128_sb = cpool.tile_from(misc128, forced_dma_engine=mybir.EngineType.Pool)
            wgt_sb = cpool.tile_from(wgt, forced_dma_engine=mybir.EngineType.Pool)
            bgt_sb = cpool.tile_from(bgt, forced_dma_engine=mybir.EngineType.Pool)
            misc8_sb = cpool.tile_from(misc8, forced_dma_engine=mybir.EngineType.Pool)
            ident_sb = cpool.tile_from(ident, forced_dma_engine=mybir.EngineType.Pool)
            # TensorScalarPtr (scalar-operand-from-AP) instructions can carry
            # only one sync wait, so scalar sources must be same-engine local:
            # stage DVE-consumed constants through a DVE copy and ACT-consumed
            # biases through an ACT copy.  After these copies each engine has
            # observed the const DMA sem once, so no later op re-waits on it.
            misc128L = cpool.tile([PD, 3 * S * KC + S], F32, name="misc128L")
            nc.vector.tensor_copy(out=misc128L[:], in_=misc128_sb[:])
            misc8L = cpool.tile([K, 2 * S * D + S], F32, name="misc8L")
            nc.vector.tensor_copy(out=misc8L[:], in_=misc8_sb[:])
            m128A = cpool.tile([PD, 3 * S * KC], F32, name="m128A")
            nc.scalar.copy(out=m128A[:], in_=misc128_sb[:, 0:3 * S * KC])
            b1tA = m128A[:, 0:S * KC]
            b2tA = m128A[:, 2 * S * KC:3 * S * KC]
            bgtA = cpool.tile([1, S], F32, name="bgtA")
            nc.scalar.copy(out=bgtA[:], in_=bgt_sb[:])
            b1t_sb = misc128L[:, 0:S * KC]
            sb1t_sb = misc128L[:, S * KC:2 * S * KC]
            b2t_sb = b2tA[:]
            slopet_sb = misc128L[:, 3 * S * KC:3 * S * KC + S]
            b1rep_sb = misc8L[:, 0:S * D]
            b2rep_sb = misc8L[:, S * D:2 * S * D]
            slope8_sb = misc8L[:, 2 * S * D:2 * S * D + S]

            # remaining weights (w1 slot 1, then w2) load behind the consts
            for kc in range(KC):
                nc.gpsimd.dma_start(out=w1_sb[1][kc][:], in_=w1[1, kc * PD:(kc + 1) * PD, :])
            for s in range(S):
                for kc in range(KC):
                    nc.gpsimd.dma_start(out=w2_sb[s][kc][:], in_=w2[s, kc * PD:(kc + 1) * PD, :])

            # ---------------- PASS A + per-b topk / pass B ----------------
            def topk_passB(b, lrow_b, cand):
                # ---- final top-k merge over the 64 per-tile candidates ----
                cand_v, cand_p = cand
                lrow = lrow_b[:]
                mx = spool.tile([S, K], F32, tag="mx")
                nc.vector.max(out=mx[:], in_=cand_v[:])
                masked = spool.tile([S, T * K], F32, tag="msk")
                nc.vector.match_replace(out=masked[:], in_to_replace=mx[:],
                                        in_values=cand_v[:], imm_value=-1e30)
                gtm = spool.tile([S, T * K], F32, tag="gtm")
                nc.vector.tensor_tensor(out=gtm[:], in0=cand_v[:], in1=masked[:],
                                        op=ALU.is_gt)
                mpk = spool.tile([S, T * K], F32, tag="mpk")
                nc.vector.tensor_tensor(out=mpk[:], in0=gtm[:], in1=cand_p[:],
                                        op=ALU.mult)
                # pk8: the 8 selected packs, descending by row index
                pk8 = spool.tile([S, K], F32, tag="pk8")
                nc.vector.max(out=pk8[:], in_=mpk[:])
                t3 = spool.tile([S, K], F32, tag="t3")
                nc.vector.tensor_scalar_mul(t3[:], pk8[:], 0.125)
                iu = spool.tile([S, K], U32, tag="iu")
                nc.vector.tensor_copy(out=iu[:], in_=t3[:])      # -> local idx+1
                tif2 = spool.tile([S, K], F32, tag="tif2")
                nc.vector.tensor_copy(out=tif2[:], in_=iu[:])
                # lhat + offset(-2) folded below; t5 = pk8 - 8*(idx+1)
                t5 = spool.tile([S, K], F32, tag="t5")
                nc.vector.scalar_tensor_tensor(
                    out=t5[:], in0=tif2[:], scalar=-8.0, in1=pk8[:],
                    op0=ALU.mult, op1=ALU.add)
                negvmax = spool.tile([S, 1], F32, tag="nvm")
                nc.vector.tensor_scalar_mul(negvmax[:], mx[:, 0:1], -1.0)
                expt = epool.tile([S, L], F32, tag="expt")
                zsum = spool.tile([S, 1], F32, tag="zsum")
                nc.scalar.activation(
                    out=expt[:], in_=lrow, func=AF.Exp,
                    bias=negvmax[:, 0:1], accum_out=zsum[:, 0:1],
                )
                recipz = spool.tile([S, 1], F32, tag="rz")
                nc.vector.reciprocal(recipz[:], zsum[:])
                # w8 = exp(lhat - vmax) * recipz, index-desc order (pass B
                # gathers rows in the same order, so any consistent order works)
                w8e = spool.tile([S, K], F32, tag="w8e")
                nc.vector.tensor_scalar(
                    out=w8e[:], in0=t5[:], scalar1=negvmax[:, 0:1],
                    scalar2=-2.0, op0=ALU.add, op1=ALU.add)
                nc.scalar.activation(out=w8e[:], in_=w8e[:], func=AF.Exp)
                w8 = spool.tile([S, K], F32, tag="w8")
                nc.vector.tensor_tensor(
                    out=w8[:], in0=w8e[:],
                    in1=recipz[:, 0:1].to_broadcast([S, K]), op=ALU.mult)
                # global row index = (local idx+1) - 1 + b*4096
                idxf = spool.tile([S, K], F32, tag="idxf")
                nc.vector.tensor_scalar_add(idxf[:], tif2[:], float(b * L - 1))
                # transpose idxf and w8 to [K, S] (rank on partitions)
                iw_ps = bps_pool.tile([K, 2 * S], F32, tag="bps")
                nc.tensor.transpose(iw_ps[:, 0:S], idxf[:], ident_sb[0:S, 0:S])
                nc.tensor.transpose(iw_ps[:, S:2 * S], w8[:], ident_sb[0:S, 0:S])
                iw_sb = spool.tile([K, 2 * S], F32, tag="iwsb")
                nc.vector.tensor_copy(out=iw_sb[:], in_=iw_ps[:])
                idxu = spool.tile([K, S], U32, tag="idxu")
                nc.vector.tensor_copy(out=idxu[:], in_=iw_sb[:, 0:S])

                # ---------------- PASS B: recompute top-8 rows ----------------
                for s in range(S):
                    q_sel = selpool.tile([K, D], F32, tag="qsel")
                    nc.gpsimd.indirect_dma_start(
                        out=q_sel[:], out_offset=None,
                        in_=qn_flat,
                        in_offset=bass.IndirectOffsetOnAxis(ap=idxu[:, s:s + 1], axis=0),
                    )
                    # q_selT chunks [128, 8] per kc
                    qst = selpool.tile([PD, KC * K], F32R, tag="qst")
                    for kc in range(KC):
                        t_ps = bps_pool.tile([PD, K], F32, tag="bps")
                        nc.tensor.transpose(
                            t_ps[:], q_sel[:, kc * PD:(kc + 1) * PD], ident_sb[0:K, 0:K])
                        nc.vector.tensor_copy(out=qst[:, kc * K:(kc + 1) * K], in_=t_ps[:])
                    # mm1 for selected rows: [8, 512]
                    hsel_ps = bps_pool.tile([K, D], F32, tag="bps2", bufs=1)
                    for kc in range(KC):
                        nc.tensor.matmul(
                            hsel_ps[:],
                            lhsT=qst[:, kc * K:(kc + 1) * K],
                            rhs=w1_sb[s][kc][:],
                            start=(kc == 0), stop=(kc == KC - 1),
                        )
                    hsel = selpool.tile([K, D], F32, tag="hsel")
                    nc.vector.tensor_tensor(
                        out=hsel[:], in0=hsel_ps[:],
                        in1=b1rep_sb[:, s * D:(s + 1) * D], op=ALU.add)
                    nc.vector.scalar_tensor_tensor(
                        out=hsel[:], in0=hsel[:], scalar=slope8_sb[:, s:s + 1],
                        in1=hsel[:], op0=ALU.mult, op1=ALU.max)
                    # transpose hsel -> [128, 8] chunks
                    hst = selpool.tile([PD, KC * K], F32R, tag="hst")
                    for kc in range(KC):
                        t_ps = bps_pool.tile([PD, K], F32, tag="bps")
                        nc.tensor.transpose(
                            t_ps[:], hsel[:, kc * PD:(kc + 1) * PD], ident_sb[0:K, 0:K])
                        nc.vector.tensor_copy(out=hst[:, kc * K:(kc + 1) * K], in_=t_ps[:])
                    xsel_ps = bps_pool.tile([K, D], F32, tag="bps2", bufs=1)
                    for kc in range(KC):
                        nc.tensor.matmul(
                            xsel_ps[:],
                            lhsT=hst[:, kc * K:(kc + 1) * K],
                            rhs=w2_sb[s][kc][:],
                            start=(kc == 0), stop=(kc == KC - 1),
                        )
                    xsel = selpool.tile([K, D], F32, tag="xsel")
                    nc.vector.tensor_tensor(
                        out=xsel[:], in0=xsel_ps[:],
                        in1=b2rep_sb[:, s * D:(s + 1) * D], op=ALU.add)
                    nc.scalar.activation(out=xsel[:], in_=xsel[:], func=AF.Tanh)
                    # combined[d] = sum_k w8[k] * xsel[k, d] -> [128, 1] per chunk
                    comb_ps = bps_pool.tile([PD, KC], F32, tag="bps")
                    for mc in range(KC):
                        nc.tensor.matmul(
                            comb_ps[:, mc:mc + 1],
                            lhsT=xsel[:, mc * PD:(mc + 1) * PD],
                            rhs=iw_sb[:, S + s:S + s + 1],
                            start=True, stop=True,
                        )
                    comb_sb = selpool.tile([PD, KC], F32, tag="combsb")
                    nc.vector.tensor_copy(out=comb_sb[:], in_=comb_ps[:])
                    nc.sync.dma_start(
                        out=out[s, :, b].rearrange("(mc p) -> p mc", p=PD),
                        in_=comb_sb[:],
                    )

            lrows = []
            cands = []
            # topk+passB for batch b is EMITTED inside batch b+1's tile loop
            # (after t==1) so its DVE/PE ops fill pipeline slack instead of
            # stalling the b->b+1 boundary; the last b runs after the loop.
            for b in range(B):
                lrow_b = lpool.tile([S, L], F32, tag="lrow", bufs=2)
                lrows.append(lrow_b)
                cand_v = lpool.tile([S, T * K], F32, tag="candv", bufs=2)
                cand_p = lpool.tile([S, T * K], F32, tag="candp", bufs=2)
                cands.append((cand_v, cand_p))
                for t in range(T):
                    # load qT tile: [128, kc x 512] (d on partitions, rows free)
                    qt_tile = qtpool.tile([PD, KC * 512], F32R, tag="qt")
                    for kc in range(KC):
                        nc.sync.dma_start(
                            out=qt_tile[:, kc * 512:(kc + 1) * 512],
                            in_=qt[b, kc * PD:(kc + 1) * PD,
                                   t * 512:(t + 1) * 512])

                    ht_tiles = []
                    # mm1 + leaky for both slots (interleaved for PE density)
                    for s in range(S):
                        ht = htpool.tile([PD, KC * 512], F32R, tag="ht")
                        ht_tiles.append(ht)
                        for mc in range(KC):
                            h_ps = hps_pool.tile([PD, 512], F32, tag="hps")
                            for kc in range(KC):
                                nc.tensor.matmul(
                                    h_ps[:],
                                    lhsT=w1_sb[s][kc][:, mc * PD:(mc + 1) * PD],
                                    rhs=qt_tile[:, kc * 512:(kc + 1) * 512],
                                    start=(kc == 0), stop=(kc == KC - 1),
                                )
                            # leaky: u = h + b1; out = max(slope*u, u).
                            # The bias add runs on ACT (Identity+bias from an
                            # ACT-local tile) to offload DVE; the DVE mult-max
                            # then carries a single ACT wait (TensorScalarPtr
                            # allows only one sync wait).
                            col = s * KC + mc
                            v = tmppool.tile([PD, 512], F32, tag="v")
                            nc.scalar.activation(
                                out=v[:], in_=h_ps[:], func=AF.Identity,
                                bias=b1tA[:, col:col + 1],
                            )
                            nc.vector.scalar_tensor_tensor(
                                out=ht[:, mc * 512:(mc + 1) * 512],
                                in0=v[:],
                                scalar=slopet_sb[:, s:s + 1],
                                in1=v[:],
                                op0=ALU.mult, op1=ALU.max,
                            )
                    xt_tiles = []
                    for s in range(S):
                        ht = ht_tiles[s]
                        xt = xtpool.tile([PD, KC * 512], F32R, tag="xt")
                        xt_tiles.append(xt)
                        for mc in range(KC):
                            x_ps = xps_pool.tile([PD, 512], F32, tag="xps")
                            for kc in range(KC):
                                nc.tensor.matmul(
                                    x_ps[:],
                                    lhsT=w2_sb[s][kc][:, mc * PD:(mc + 1) * PD],
                                    rhs=ht[:, kc * 512:(kc + 1) * 512],
                                    start=(kc == 0), stop=(kc == KC - 1),
                                )
                            col = s * KC + mc
                            nc.scalar.activation(
                                out=xt[:, mc * 512:(mc + 1) * 512], in_=x_ps[:],
                                func=AF.Tanh, bias=b2t_sb[:, col:col + 1],
                            )
                    # gate matvec per slot ([1, 512] psum each; PE out must
                    # start at partition 0). Engine ops can only address SBUF
                    # partitions 0/32/64/96, so stage the row at partition 0
                    # (with +bg) and DMA it into lrow_b's partition s.
                    for s in range(S):
                        g_ps = gps_pool.tile([1, 512], F32, tag="gps")
                        for kc in range(KC):
                            nc.tensor.matmul(
                                g_ps[:],
                                lhsT=wgt_sb[:, s * KC + kc:s * KC + kc + 1],
                                rhs=xt_tiles[s][:, kc * 512:(kc + 1) * 512],
                                start=(kc == 0), stop=(kc == KC - 1),
                            )
                        lstage = tmppool.tile([1, 512], F32, tag="lstage")
                        nc.scalar.activation(
                            out=lstage[:], in_=g_ps[:], func=AF.Identity,
                            bias=bgtA[0:1, s:s + 1],
                        )
                        nc.sync.dma_start(
                            out=lrow_b[s:s + 1, t * 512:(t + 1) * 512],
                            in_=lstage[:])

                    # hierarchical top-k, overlapped stage: per-tile top-8
                    # values + packed (index, clamped value) candidates.
                    # pack = 8*(local_idx+1) + (clamp(l, +-1.9) + 2): integer
                    # part recovers the index under truncation OR
                    # round-to-nearest; the fraction carries the logit to
                    # ~4e-3, used only for softmax weights (selection uses
                    # exact values in cand_v).
                    lslice = lrow_b[:, t * 512:(t + 1) * 512]
                    nc.vector.max(out=cand_v[:, t * K:(t + 1) * K], in_=lslice)
                    ti = spool.tile([S, K], U32, tag="ti")
                    nc.vector.max_index(out=ti[:], in_max=cand_v[:, t * K:(t + 1) * K],
                                        in_values=lslice)
                    tif = spool.tile([S, K], F32, tag="tif")
                    nc.vector.tensor_copy(out=tif[:], in_=ti[:])
                    t1 = spool.tile([S, K], F32, tag="t1")
                    nc.vector.tensor_scalar(
                        out=t1[:], in0=tif[:], scalar1=float(t * 512 + 1),
                        scalar2=8.0, op0=ALU.add, op1=ALU.mult)
                    t2 = spool.tile([S, K], F32, tag="t2")
                    nc.vector.tensor_scalar(
                        out=t2[:], in0=cand_v[:, t * K:(t + 1) * K],
                        scalar1=1.9, scalar2=-1.9, op0=ALU.min, op1=ALU.max)
                    nc.vector.scalar_tensor_tensor(
                        out=cand_p[:, t * K:(t + 1) * K], in0=t2[:], scalar=2.0,
                        in1=t1[:], op0=ALU.add, op1=ALU.add)

                    if t == 1 and b > 0:
                        topk_passB(b - 1, lrows[b - 1], cands[b - 1])


            topk_passB(B - 1, lrows[B - 1], cands[B - 1])

    nc.compile()  # Bacc passes: reg alloc, DCE, wait splitting (TRN2 1-wait rule)
    _PROGRAM_CACHE["nc"] = nc
    return nc


def _prep_in_maps(query, W1, b1, W2, b2, Wg, bg):
    query = np.ascontiguousarray(query, dtype=np.float32)
    qt = np.ascontiguousarray(query.transpose(0, 2, 1))
    slopes = (0.01 + 0.2 / N_MEM * np.arange(N_MEM, dtype=np.float32))
    ident = np.eye(PD, dtype=np.float32)
    in_maps = []
    for c in range(NCORES):
        sl = slice(S * c, S * (c + 1))
        b1c = np.asarray(b1[sl], np.float32)     # [S, D]
        b2c = np.asarray(b2[sl], np.float32)
        wgc = np.asarray(Wg[sl], np.float32)
        slc = slopes[sl]                          # [S]
        sb1c = slc[:, None] * b1c
        def t128(a):  # [S, D] -> [128, S*KC]
            return np.ascontiguousarray(
                a.reshape(S, KC, PD).transpose(2, 0, 1).reshape(PD, S * KC))
        misc128 = np.concatenate([
            t128(b1c), t128(sb1c), t128(b2c),
            np.broadcast_to(slc[None, :], (PD, S)),
        ], axis=1).astype(np.float32)
        misc8 = np.concatenate([
            np.broadcast_to(b1c.reshape(1, S * D), (K, S * D)),
            np.broadcast_to(b2c.reshape(1, S * D), (K, S * D)),
            np.broadcast_to(slc[None, :], (K, S)),
        ], axis=1).astype(np.float32)
        in_maps.append({
            "qt": qt,
            "qn": query,
            "w1": np.ascontiguousarray(W1[sl], np.float32),
            "w2": np.ascontiguousarray(W2[sl], np.float32),
            "misc128": np.ascontiguousarray(misc128),
            "wgt": t128(wgc),
            "bgt": np.asarray(bg[sl], np.float32).reshape(1, S),
            "misc8": np.ascontiguousarray(misc8),
            "ident": ident,
        })
    return in_maps


def kernel(query, W1, b1, W2, b2, Wg, bg, topk, **_ignored):
    assert int(topk) == K, f"kernel hardcodes topk={K}, got {topk}"
    nc = _build_program()
    in_maps = _prep_in_maps(
        np.asarray(query), np.asarray(W1), np.asarray(b1), np.asarray(W2),
        np.asarray(b2), np.asarray(Wg), np.asarray(bg))
    res = bass_utils.run_bass_kernel_spmd(nc, in_maps, core_ids=list(range(NCORES)))
    outs = res.results
    # outs[c]["out"] is [S, D, B]; assemble [B, N_MEM, D]
    m = np.empty((B, N_MEM, D), dtype=np.float32)
    for c in range(NCORES):
        oc = np.asarray(outs[c]["out"])  # [S, D, B]
        for s in range(S):
            m[:, S * c + s, :] = oc[s].T
    norm = np.maximum(np.linalg.norm(m.astype(np.float64), axis=-1, keepdims=True),
                      1e-12).astype(np.float32)
    return (m / norm).astype(np.float32)


# revision 24
# speedup vs baseline: 1.0853x; 1.0138x over previous
"""Trainium2 Bass kernel for the topk_masking memory-module problem.

Computation (reference semantics):
  For each of n=16 memory slots l:
    h = LeakyReLU_{slope_l}(q @ W1[l] + b1[l])          # [b, L, d]
    x = tanh(h @ W2[l] + b2[l])                          # [b, L, d]
    logits = x @ Wg[l] + bg[l]                           # [b, L]
    w = softmax(logits over L); top8 (values+indices)
    combined[b] = sum_k w_topk[k] * x[b, idx_k]          # [b, d]
  out[b, l, :] = normalize(combined over d)

Sharding: expert-parallel over the 16 memory slots -> 2 slots per core on
8 cores.  Each core runs the full [4, 4096, 512] query through its two
slots.  Device does everything except the final L2 normalize (host, cheap).

Device algorithm per core (pass A computes logits while discarding x; the
top-8 rows of x are recomputed in pass B from the gathered q rows):
  pass A: for b, for t (8 row-tiles of 512):
    hT = leaky(W1^T-chunks @ qT-tile + b1)   (transposed pipeline, d on
    xT = tanh(W2-chunks @ hT + b2)            partitions, rows on free)
    logits[2b+s, t*512:] = Wg . xT + bg      (PE matvec)
  per b: max8 + max_index give top-8 values+indices (paired, descending);
    softmax stats via Exp activation with accumulate; weights from values.
  pass B per (b, s): indirect-DMA gather the 8 q rows, recompute their x
    (tiny matmuls), then combined = x_sel^T @ w8 on PE; DMA to out[s,:,b].
"""

import numpy as np

import concourse.bass as bass
import concourse.bacc as bacc
import concourse.mybir as mybir
from concourse import bass_utils
from concourse.tile import TileContext

F32 = mybir.dt.float32
F32R = mybir.dt.float32r
U32 = mybir.dt.uint32
AF = mybir.ActivationFunctionType
ALU = mybir.AluOpType

B = 4
L = 4096
D = 512
N_MEM = 16
NCORES = 8
S = N_MEM // NCORES  # 2 slots per core
K = 8
T = L // 512  # 8 row-tiles per batch
PD = 128     # partition dim
KC = D // PD  # 4 contraction chunks

# The heavy pipeline runs the PE in float32r (fp32 operands, 1 cycle/row vs
# 4 for plain float32; slightly reduced multiply precision).  Top-k selection
# is sensitive to logit error, so this choice is validated against the
# reference in test.py on both CPU- and axon-generated datasets.
_PROGRAM_CACHE = {}


def _build_program():
    if "nc" in _PROGRAM_CACHE:
        return _PROGRAM_CACHE["nc"]

    nc = bacc.Bacc("TRN2", debug=False, enable_asserts=False, num_devices=NCORES)

    qt = nc.dram_tensor("qt", [B, D, L], F32R, kind="ExternalInput").ap()
    qn = nc.dram_tensor("qn", [B, L, D], F32, kind="ExternalInput").ap()
    w1 = nc.dram_tensor("w1", [S, D, D], F32R, kind="ExternalInput").ap()
    w2 = nc.dram_tensor("w2", [S, D, D], F32R, kind="ExternalInput").ap()
    # small constants packed into two tensors (one DMA each) so consumers
    # carry few semaphore waits: misc128 = [b1t | sb1t | b2t | slopet],
    # misc8 = [b1rep | b2rep | slope8]
    misc128 = nc.dram_tensor("misc128", [PD, 3 * S * KC + S], F32,
                             kind="ExternalInput").ap()
    wgt = nc.dram_tensor("wgt", [PD, S * KC], F32R, kind="ExternalInput").ap()
    bgt = nc.dram_tensor("bgt", [1, S], F32, kind="ExternalInput").ap()
    misc8 = nc.dram_tensor("misc8", [K, 2 * S * D + S], F32,
                           kind="ExternalInput").ap()
    ident = nc.dram_tensor("ident", [PD, PD], F32, kind="ExternalInput").ap()
    out = nc.dram_tensor("out", [S, D, B], F32, kind="ExternalOutput").ap()

    qn_flat = qn.rearrange("b l d -> (b l) d")

    with TileContext(nc) as tc:
        with (
            tc.tile_pool(name="consts", bufs=1) as cpool,
            tc.tile_pool(name="weights", bufs=1) as wpool,
            tc.tile_pool(name="qtp", bufs=2) as qtpool,
            tc.tile_pool(name="ht", bufs=3) as htpool,
            tc.tile_pool(name="xt", bufs=3) as xtpool,
            tc.tile_pool(name="tmp", bufs=3) as tmppool,
            tc.tile_pool(name="logits", bufs=1) as lpool,
            tc.tile_pool(name="small", bufs=8) as spool,
            tc.tile_pool(name="expp", bufs=1) as epool,
            tc.tile_pool(name="selp", bufs=2) as selpool,
            tc.tile_pool(name="hps", bufs=3, space="PSUM") as hps_pool,
            tc.tile_pool(name="xps", bufs=3, space="PSUM") as xps_pool,
            tc.tile_pool(name="gps", bufs=1, space="PSUM") as gps_pool,
            tc.tile_pool(name="bps", bufs=1, space="PSUM") as bps_pool,
        ):
            # --- weights for mm1 slot 0 first: the PE's first matmul only
            # needs w1[0][0] + the first qt tile (sync queue), so their DMAs
            # lead both queues ---
            w1_sb = [[wpool.tile([PD, D], F32R, name=f"w1sb_{s}_{kc}", tag=f"w1_{s}_{kc}")
                      for kc in range(KC)] for s in range(S)]
            w2_sb = [[wpool.tile([PD, D], F32R, name=f"w2sb_{s}_{kc}", tag=f"w2_{s}_{kc}")
                      for kc in range(KC)] for s in range(S)]
            for kc in range(KC):
                nc.gpsimd.dma_start(out=w1_sb[0][kc][:], in_=w1[0, kc * PD:(kc + 1) * PD, :])

            # --- persistent constants / weights in SBUF ---
            misc128_sb = cpool.tile_from(misc128, forced_dma_engine=mybir.EngineType.Pool)
            wgt_sb = cpool.tile_from(wgt, forced_dma_engine=mybir.EngineType.Pool)
            bgt_sb = cpool.tile_from(bgt, forced_dma_engine=mybir.EngineType.Pool)
            misc8_sb = cpool.tile_from(misc8, forced_dma_engine=mybir.EngineType.Pool)
            ident_sb = cpool.tile_from(ident, forced_dma_engine=mybir.EngineType.Pool)
            # TensorScalarPtr (scalar-operand-from-AP) instructions can carry
            # only one sync wait, so scalar sources must be same-engine local:
            # stage DVE-consumed constants through a DVE copy and ACT-consumed
            # biases through an ACT copy.  After these copies each engine has
            # observed the const DMA sem once, so no later op re-waits on it.
            misc128L = cpool.tile([PD, 3 * S * KC + S], F32, name="misc128L")
            nc.vector.tensor_copy(out=misc128L[:], in_=misc128_sb[:])
            misc8L = cpool.tile([K, 2 * S * D + S], F32, name="misc8L")
            nc.vector.tensor_copy(out=misc8L[:], in_=misc8_sb[:])
            m128A = cpool.tile([PD, 3 * S * KC], F32, name="m128A")
            nc.scalar.copy(out=m128A[:], in_=misc128_sb[:, 0:3 * S * KC])
            b1tA = m128A[:, 0:S * KC]
            b2tA = m128A[:, 2 * S * KC:3 * S * KC]
            bgtA = cpool.tile([1, S], F32, name="bgtA")
            nc.scalar.copy(out=bgtA[:], in_=bgt_sb[:])
            b1t_sb = misc128L[:, 0:S * KC]
            sb1t_sb = misc128L[:, S * KC:2 * S * KC]
            b2t_sb = b2tA[:]
            slopet_sb = misc128L[:, 3 * S * KC:3 * S * KC + S]
            b1rep_sb = misc8L[:, 0:S * D]
            b2rep_sb = misc8L[:, S * D:2 * S * D]
            slope8_sb = misc8L[:, 2 * S * D:2 * S * D + S]

            # remaining weights (w1 slot 1, then w2) load behind the consts
            for kc in range(KC):
                nc.gpsimd.dma_start(out=w1_sb[1][kc][:], in_=w1[1, kc * PD:(kc + 1) * PD, :])
            for s in range(S):
                for kc in range(KC):
                    nc.gpsimd.dma_start(out=w2_sb[s][kc][:], in_=w2[s, kc * PD:(kc + 1) * PD, :])

            # ---------------- PASS A + per-b topk / pass B ----------------
            def topk_passB(b, lrow_b, cand):
                # ---- final top-k merge over the 64 per-tile candidates ----
                cand_v, cand_p = cand
                lrow = lrow_b[:]
                mx = spool.tile([S, K], F32, tag="mx")
                nc.vector.max(out=mx[:], in_=cand_v[:])
                masked = spool.tile([S, T * K], F32, tag="msk")
                nc.vector.match_replace(out=masked[:], in_to_replace=mx[:],
                                        in_values=cand_v[:], imm_value=-1e30)
                gtm = spool.tile([S, T * K], F32, tag="gtm")
                nc.vector.tensor_tensor(out=gtm[:], in0=cand_v[:], in1=masked[:],
                                        op=ALU.is_gt)
                mpk = spool.tile([S, T * K], F32, tag="mpk")
                nc.vector.tensor_tensor(out=mpk[:], in0=gtm[:], in1=cand_p[:],
                                        op=ALU.mult)
                # pk8: the 8 selected packs, descending by row index
                pk8 = spool.tile([S, K], F32, tag="pk8")
                nc.vector.max(out=pk8[:], in_=mpk[:])
                t3 = spool.tile([S, K], F32, tag="t3")
                nc.vector.tensor_scalar_mul(t3[:], pk8[:], 0.125)
                iu = spool.tile([S, K], U32, tag="iu")
                nc.vector.tensor_copy(out=iu[:], in_=t3[:])      # -> local idx+1
                tif2 = spool.tile([S, K], F32, tag="tif2")
                nc.vector.tensor_copy(out=tif2[:], in_=iu[:])
                # lhat + offset(-2) folded below; t5 = pk8 - 8*(idx+1)
                t5 = spool.tile([S, K], F32, tag="t5")
                nc.vector.scalar_tensor_tensor(
                    out=t5[:], in0=tif2[:], scalar=-8.0, in1=pk8[:],
                    op0=ALU.mult, op1=ALU.add)
                negvmax = spool.tile([S, 1], F32, tag="nvm")
                nc.vector.tensor_scalar_mul(negvmax[:], mx[:, 0:1], -1.0)
                expt = epool.tile([S, L], F32, tag="expt")
                zsum = spool.tile([S, 1], F32, tag="zsum")
                nc.scalar.activation(
                    out=expt[:], in_=lrow, func=AF.Exp,
                    bias=negvmax[:, 0:1], accum_out=zsum[:, 0:1],
                )
                recipz = spool.tile([S, 1], F32, tag="rz")
                nc.vector.reciprocal(recipz[:], zsum[:])
                # w8 = exp(lhat - vmax) * recipz, index-desc order (pass B
                # gathers rows in the same order, so any consistent order works)
                w8e = spool.tile([S, K], F32, tag="w8e")
                nc.vector.tensor_scalar(
                    out=w8e[:], in0=t5[:], scalar1=negvmax[:, 0:1],
                    scalar2=-2.0, op0=ALU.add, op1=ALU.add)
                nc.scalar.activation(out=w8e[:], in_=w8e[:], func=AF.Exp)
                w8 = spool.tile([S, K], F32, tag="w8")
                nc.vector.tensor_tensor(
                    out=w8[:], in0=w8e[:],
                    in1=recipz[:, 0:1].to_broadcast([S, K]), op=ALU.mult)
                # global row index = (local idx+1) - 1 + b*4096
                idxf = spool.tile([S, K], F32, tag="idxf")
                nc.vector.tensor_scalar_add(idxf[:], tif2[:], float(b * L - 1))
                # transpose idxf and w8 to [K, S] (rank on partitions)
                iw_ps = bps_pool.tile([K, 2 * S], F32, tag="bps", bufs=1)
                nc.tensor.transpose(iw_ps[:, 0:S], idxf[:], ident_sb[0:S, 0:S])
                nc.tensor.transpose(iw_ps[:, S:2 * S], w8[:], ident_sb[0:S, 0:S])
                iw_sb = spool.tile([K, 2 * S], F32, tag="iwsb")
                nc.vector.tensor_copy(out=iw_sb[:], in_=iw_ps[:])
                idxu = spool.tile([K, S], U32, tag="idxu")
                nc.vector.tensor_copy(out=idxu[:], in_=iw_sb[:, 0:S])

                # ---------------- PASS B: recompute top-8 rows ----------------
                for s in range(S):
                    q_sel = selpool.tile([K, D], F32, tag="qsel")
                    nc.gpsimd.indirect_dma_start(
                        out=q_sel[:], out_offset=None,
                        in_=qn_flat,
                        in_offset=bass.IndirectOffsetOnAxis(ap=idxu[:, s:s + 1], axis=0),
                    )
                    # q_selT chunks [128, 8] per kc
                    qst = selpool.tile([PD, KC * K], F32R, tag="qst")
                    for kc in range(KC):
                        t_ps = bps_pool.tile([PD, K], F32, tag="bps", bufs=1)
                        nc.tensor.transpose(
                            t_ps[:], q_sel[:, kc * PD:(kc + 1) * PD], ident_sb[0:K, 0:K])
                        nc.vector.tensor_copy(out=qst[:, kc * K:(kc + 1) * K], in_=t_ps[:])
                    # mm1 for selected rows: [8, 512]
                    hsel_ps = bps_pool.tile([K, D], F32, tag="bps", bufs=1)
                    for kc in range(KC):
                        nc.tensor.matmul(
                            hsel_ps[:],
                            lhsT=qst[:, kc * K:(kc + 1) * K],
                            rhs=w1_sb[s][kc][:],
                            start=(kc == 0), stop=(kc == KC - 1),
                        )
                    hsel = selpool.tile([K, D], F32, tag="hsel")
                    nc.vector.tensor_tensor(
                        out=hsel[:], in0=hsel_ps[:],
                        in1=b1rep_sb[:, s * D:(s + 1) * D], op=ALU.add)
                    nc.vector.scalar_tensor_tensor(
                        out=hsel[:], in0=hsel[:], scalar=slope8_sb[:, s:s + 1],
                        in1=hsel[:], op0=ALU.mult, op1=ALU.max)
                    # transpose hsel -> [128, 8] chunks
                    hst = selpool.tile([PD, KC * K], F32R, tag="hst")
                    for kc in range(KC):
                        t_ps = bps_pool.tile([PD, K], F32, tag="bps", bufs=1)
                        nc.tensor.transpose(
                            t_ps[:], hsel[:, kc * PD:(kc + 1) * PD], ident_sb[0:K, 0:K])
                        nc.vector.tensor_copy(out=hst[:, kc * K:(kc + 1) * K], in_=t_ps[:])
                    xsel_ps = bps_pool.tile([K, D], F32, tag="bps", bufs=1)
                    for kc in range(KC):
                        nc.tensor.matmul(
                            xsel_ps[:],
                            lhsT=hst[:, kc * K:(kc + 1) * K],
                            rhs=w2_sb[s][kc][:],
                            start=(kc == 0), stop=(kc == KC - 1),
                        )
                    xsel = selpool.tile([K, D], F32, tag="xsel")
                    nc.vector.tensor_tensor(
                        out=xsel[:], in0=xsel_ps[:],
                        in1=b2rep_sb[:, s * D:(s + 1) * D], op=ALU.add)
                    nc.scalar.activation(out=xsel[:], in_=xsel[:], func=AF.Tanh)
                    # combined[d] = sum_k w8[k] * xsel[k, d] -> [128, 1] per chunk
                    comb_ps = bps_pool.tile([PD, KC], F32, tag="bps", bufs=1)
                    for mc in range(KC):
                        nc.tensor.matmul(
                            comb_ps[:, mc:mc + 1],
                            lhsT=xsel[:, mc * PD:(mc + 1) * PD],
                            rhs=iw_sb[:, S + s:S + s + 1],
                            start=True, stop=True,
                        )
                    comb_sb = selpool.tile([PD, KC], F32, tag="combsb")
                    nc.vector.tensor_copy(out=comb_sb[:], in_=comb_ps[:])
                    nc.sync.dma_start(
                        out=out[s, :, b].rearrange("(mc p) -> p mc", p=PD),
                        in_=comb_sb[:],
                    )

            lrows = []
            cands = []
            # topk+passB for batch b is EMITTED inside batch b+1's tile loop
            # (after t==1) so its DVE/PE ops fill pipeline slack instead of
            # stalling the b->b+1 boundary; the last b runs after the loop.
            for b in range(B):
                lrow_b = lpool.tile([S, L], F32, tag="lrow", bufs=2)
                lrows.append(lrow_b)
                cand_v = lpool.tile([S, T * K], F32, tag="candv", bufs=2)
                cand_p = lpool.tile([S, T * K], F32, tag="candp", bufs=2)
                cands.append((cand_v, cand_p))
                for t in range(T):
                    # load qT tile: [128, kc x 512] (d on partitions, rows free)
                    qt_tile = qtpool.tile([PD, KC * 512], F32R, tag="qt")
                    for kc in range(KC):
                        nc.sync.dma_start(
                            out=qt_tile[:, kc * 512:(kc + 1) * 512],
                            in_=qt[b, kc * PD:(kc + 1) * PD,
                                   t * 512:(t + 1) * 512])

                    ht_tiles = []
                    # mm1 + leaky for both slots (interleaved for PE density)
                    for s in range(S):
                        ht = htpool.tile([PD, KC * 512], F32R, tag="ht")
                        ht_tiles.append(ht)
                        for mc in range(KC):
                            h_ps = hps_pool.tile([PD, 512], F32, tag="hps")
                            for kc in range(KC):
                                nc.tensor.matmul(
                                    h_ps[:],
                                    lhsT=w1_sb[s][kc][:, mc * PD:(mc + 1) * PD],
                                    rhs=qt_tile[:, kc * 512:(kc + 1) * 512],
                                    start=(kc == 0), stop=(kc == KC - 1),
                                )
                            # leaky: u = h + b1; out = max(slope*u, u).
                            # The bias add runs on ACT (Identity+bias from an
                            # ACT-local tile) to offload DVE; the DVE mult-max
                            # then carries a single ACT wait (TensorScalarPtr
                            # allows only one sync wait).
                            col = s * KC + mc
                            v = tmppool.tile([PD, 512], F32, tag="v")
                            nc.scalar.activation(
                                out=v[:], in_=h_ps[:], func=AF.Identity,
                                bias=b1tA[:, col:col + 1],
                            )
                            nc.vector.scalar_tensor_tensor(
                                out=ht[:, mc * 512:(mc + 1) * 512],
                                in0=v[:],
                                scalar=slopet_sb[:, s:s + 1],
                                in1=v[:],
                                op0=ALU.mult, op1=ALU.max,
                            )
                    xt_tiles = []
                    for s in range(S):
                        ht = ht_tiles[s]
                        xt = xtpool.tile([PD, KC * 512], F32R, tag="xt")
                        xt_tiles.append(xt)
                        for mc in range(KC):
                            x_ps = xps_pool.tile([PD, 512], F32, tag="xps")
                            for kc in range(KC):
                                nc.tensor.matmul(
                                    x_ps[:],
                                    lhsT=w2_sb[s][kc][:, mc * PD:(mc + 1) * PD],
                                    rhs=ht[:, kc * 512:(kc + 1) * 512],
                                    start=(kc == 0), stop=(kc == KC - 1),
                                )
                            col = s * KC + mc
                            nc.scalar.activation(
                                out=xt[:, mc * 512:(mc + 1) * 512], in_=x_ps[:],
                                func=AF.Tanh, bias=b2t_sb[:, col:col + 1],
                            )
                    # gate matvec per slot ([1, 512] psum each; PE out must
                    # start at partition 0). Engine ops can only address SBUF
                    # partitions 0/32/64/96, so stage the row at partition 0
                    # (with +bg) and DMA it into lrow_b's partition s.
                    for s in range(S):
                        g_ps = gps_pool.tile([1, 512], F32, tag="gps")
                        for kc in range(KC):
                            nc.tensor.matmul(
                                g_ps[:],
                                lhsT=wgt_sb[:, s * KC + kc:s * KC + kc + 1],
                                rhs=xt_tiles[s][:, kc * 512:(kc + 1) * 512],
                                start=(kc == 0), stop=(kc == KC - 1),
                            )
                        lstage = tmppool.tile([1, 512], F32, tag="lstage")
                        nc.scalar.activation(
                            out=lstage[:], in_=g_ps[:], func=AF.Identity,
                            bias=bgtA[0:1, s:s + 1],
                        )
                        nc.sync.dma_start(
                            out=lrow_b[s:s + 1, t * 512:(t + 1) * 512],
                            in_=lstage[:])

                    # hierarchical top-k, overlapped stage: per-tile top-8
                    # values + packed (index, clamped value) candidates.
                    # pack = 8*(local_idx+1) + (clamp(l, +-1.9) + 2): integer
                    # part recovers the index under truncation OR
                    # round-to-nearest; the fraction carries the logit to
                    # ~4e-3, used only for softmax weights (selection uses
                    # exact values in cand_v).
                    lslice = lrow_b[:, t * 512:(t + 1) * 512]
                    nc.vector.max(out=cand_v[:, t * K:(t + 1) * K], in_=lslice)
                    ti = spool.tile([S, K], U32, tag="ti")
                    nc.vector.max_index(out=ti[:], in_max=cand_v[:, t * K:(t + 1) * K],
                                        in_values=lslice)
                    tif = spool.tile([S, K], F32, tag="tif")
                    nc.vector.tensor_copy(out=tif[:], in_=ti[:])
                    t1 = spool.tile([S, K], F32, tag="t1")
                    nc.vector.tensor_scalar(
                        out=t1[:], in0=tif[:], scalar1=float(t * 512 + 1),
                        scalar2=8.0, op0=ALU.add, op1=ALU.mult)
                    t2 = spool.tile([S, K], F32, tag="t2")
                    nc.vector.tensor_scalar(
                        out=t2[:], in0=cand_v[:, t * K:(t + 1) * K],
                        scalar1=1.9, scalar2=-1.9, op0=ALU.min, op1=ALU.max)
                    nc.vector.scalar_tensor_tensor(
                        out=cand_p[:, t * K:(t + 1) * K], in0=t2[:], scalar=2.0,
                        in1=t1[:], op0=ALU.add, op1=ALU.add)

                    if t == 1 and b > 0:
                        topk_passB(b - 1, lrows[b - 1], cands[b - 1])


            topk_passB(B - 1, lrows[B - 1], cands[B - 1])

    nc.compile()  # Bacc passes: reg alloc, DCE, wait splitting (TRN2 1-wait rule)
    _PROGRAM_CACHE["nc"] = nc
    return nc


def _prep_in_maps(query, W1, b1, W2, b2, Wg, bg):
    query = np.ascontiguousarray(query, dtype=np.float32)
    qt = np.ascontiguousarray(query.transpose(0, 2, 1))
    slopes = (0.01 + 0.2 / N_MEM * np.arange(N_MEM, dtype=np.float32))
    ident = np.eye(PD, dtype=np.float32)
    in_maps = []
    for c in range(NCORES):
        sl = slice(S * c, S * (c + 1))
        b1c = np.asarray(b1[sl], np.float32)     # [S, D]
        b2c = np.asarray(b2[sl], np.float32)
        wgc = np.asarray(Wg[sl], np.float32)
        slc = slopes[sl]                          # [S]
        sb1c = slc[:, None] * b1c
        def t128(a):  # [S, D] -> [128, S*KC]
            return np.ascontiguousarray(
                a.reshape(S, KC, PD).transpose(2, 0, 1).reshape(PD, S * KC))
        misc128 = np.concatenate([
            t128(b1c), t128(sb1c), t128(b2c),
            np.broadcast_to(slc[None, :], (PD, S)),
        ], axis=1).astype(np.float32)
        misc8 = np.concatenate([
            np.broadcast_to(b1c.reshape(1, S * D), (K, S * D)),
            np.broadcast_to(b2c.reshape(1, S * D), (K, S * D)),
            np.broadcast_to(slc[None, :], (K, S)),
        ], axis=1).astype(np.float32)
        in_maps.append({
            "qt": qt,
            "qn": query,
            "w1": np.ascontiguousarray(W1[sl], np.float32),
            "w2": np.ascontiguousarray(W2[sl], np.float32),
            "misc128": np.ascontiguousarray(misc128),
            "wgt": t128(wgc),
            "bgt": np.asarray(bg[sl], np.float32).reshape(1, S),
            "misc8": np.ascontiguousarray(misc8),
            "ident": ident,
        })
    return in_maps


def kernel(query, W1, b1, W2, b2, Wg, bg, topk, **_ignored):
    assert int(topk) == K, f"kernel hardcodes topk={K}, got {topk}"
    nc = _build_program()
    in_maps = _prep_in_maps(
        np.asarray(query), np.asarray(W1), np.asarray(b1), np.asarray(W2),
        np.asarray(b2), np.asarray(Wg), np.asarray(bg))
    res = bass_utils.run_bass_kernel_spmd(nc, in_maps, core_ids=list(range(NCORES)))
    outs = res.results
    # outs[c]["out"] is [S, D, B]; assemble [B, N_MEM, D]
    m = np.empty((B, N_MEM, D), dtype=np.float32)
    for c in range(NCORES):
        oc = np.asarray(outs[c]["out"])  # [S, D, B]
        for s in range(S):
            m[:, S * c + s, :] = oc[s].T
    norm = np.maximum(np.linalg.norm(m.astype(np.float64), axis=-1, keepdims=True),
                      1e-12).astype(np.float32)
    return (m / norm).astype(np.float32)


# revision 25
# speedup vs baseline: 1.0873x; 1.0019x over previous
"""Trainium2 Bass kernel for the topk_masking memory-module problem.

Computation (reference semantics):
  For each of n=16 memory slots l:
    h = LeakyReLU_{slope_l}(q @ W1[l] + b1[l])          # [b, L, d]
    x = tanh(h @ W2[l] + b2[l])                          # [b, L, d]
    logits = x @ Wg[l] + bg[l]                           # [b, L]
    w = softmax(logits over L); top8 (values+indices)
    combined[b] = sum_k w_topk[k] * x[b, idx_k]          # [b, d]
  out[b, l, :] = normalize(combined over d)

Sharding: expert-parallel over the 16 memory slots -> 2 slots per core on
8 cores.  Each core runs the full [4, 4096, 512] query through its two
slots.  Device does everything except the final L2 normalize (host, cheap).

Device algorithm per core (pass A computes logits while discarding x; the
top-8 rows of x are recomputed in pass B from the gathered q rows):
  pass A: for b, for t (8 row-tiles of 512):
    hT = leaky(W1^T-chunks @ qT-tile + b1)   (transposed pipeline, d on
    xT = tanh(W2-chunks @ hT + b2)            partitions, rows on free)
    logits[2b+s, t*512:] = Wg . xT + bg      (PE matvec)
  per b: max8 + max_index give top-8 values+indices (paired, descending);
    softmax stats via Exp activation with accumulate; weights from values.
  pass B per (b, s): indirect-DMA gather the 8 q rows, recompute their x
    (tiny matmuls), then combined = x_sel^T @ w8 on PE; DMA to out[s,:,b].
"""

import numpy as np

import concourse.bass as bass
import concourse.bacc as bacc
import concourse.mybir as mybir
from concourse import bass_utils
from concourse.tile import TileContext

F32 = mybir.dt.float32
F32R = mybir.dt.float32r
U32 = mybir.dt.uint32
AF = mybir.ActivationFunctionType
ALU = mybir.AluOpType

B = 4
L = 4096
D = 512
N_MEM = 16
NCORES = 8
S = N_MEM // NCORES  # 2 slots per core
K = 8
T = L // 512  # 8 row-tiles per batch
PD = 128     # partition dim
KC = D // PD  # 4 contraction chunks

# The heavy pipeline runs the PE in float32r (fp32 operands, 1 cycle/row vs
# 4 for plain float32; slightly reduced multiply precision).  Top-k selection
# is sensitive to logit error, so this choice is validated against the
# reference in test.py on both CPU- and axon-generated datasets.
_PROGRAM_CACHE = {}


def _build_program():
    if "nc" in _PROGRAM_CACHE:
        return _PROGRAM_CACHE["nc"]

    nc = bacc.Bacc("TRN2", debug=False, enable_asserts=False, num_devices=NCORES)

    qt = nc.dram_tensor("qt", [B, D, L], F32R, kind="ExternalInput").ap()
    qn = nc.dram_tensor("qn", [B, L, D], F32, kind="ExternalInput").ap()
    w1 = nc.dram_tensor("w1", [S, D, D], F32R, kind="ExternalInput").ap()
    w2 = nc.dram_tensor("w2", [S, D, D], F32R, kind="ExternalInput").ap()
    # small constants packed into two tensors (one DMA each) so consumers
    # carry few semaphore waits: misc128 = [b1t | sb1t | b2t | slopet],
    # misc8 = [b1rep | b2rep | slope8]
    misc128 = nc.dram_tensor("misc128", [PD, 3 * S * KC + S], F32,
                             kind="ExternalInput").ap()
    wgt = nc.dram_tensor("wgt", [PD, S * KC], F32R, kind="ExternalInput").ap()
    bgt = nc.dram_tensor("bgt", [1, S], F32, kind="ExternalInput").ap()
    misc8 = nc.dram_tensor("misc8", [K, 2 * S * D + S], F32,
                           kind="ExternalInput").ap()
    ident = nc.dram_tensor("ident", [PD, PD], F32, kind="ExternalInput").ap()
    out = nc.dram_tensor("out", [S, D, B], F32, kind="ExternalOutput").ap()

    qn_flat = qn.rearrange("b l d -> (b l) d")

    with TileContext(nc) as tc:
        with (
            tc.tile_pool(name="consts", bufs=1) as cpool,
            tc.tile_pool(name="weights", bufs=1) as wpool,
            tc.tile_pool(name="qtp", bufs=2) as qtpool,
            tc.tile_pool(name="ht", bufs=3) as htpool,
            tc.tile_pool(name="xt", bufs=3) as xtpool,
            tc.tile_pool(name="tmp", bufs=3) as tmppool,
            tc.tile_pool(name="logits", bufs=1) as lpool,
            tc.tile_pool(name="small", bufs=8) as spool,
            tc.tile_pool(name="expp", bufs=1) as epool,
            tc.tile_pool(name="selp", bufs=2) as selpool,
            tc.tile_pool(name="hps", bufs=3, space="PSUM") as hps_pool,
            tc.tile_pool(name="xps", bufs=3, space="PSUM") as xps_pool,
            tc.tile_pool(name="gps", bufs=1, space="PSUM") as gps_pool,
            tc.tile_pool(name="bps", bufs=1, space="PSUM") as bps_pool,
        ):
            # --- weights for mm1 slot 0 first: the PE's first matmul only
            # needs w1[0][0] + the first qt tile (sync queue), so their DMAs
            # lead both queues ---
            w1_sb = [[wpool.tile([PD, D], F32R, name=f"w1sb_{s}_{kc}", tag=f"w1_{s}_{kc}")
                      for kc in range(KC)] for s in range(S)]
            w2_sb = [[wpool.tile([PD, D], F32R, name=f"w2sb_{s}_{kc}", tag=f"w2_{s}_{kc}")
                      for kc in range(KC)] for s in range(S)]
            for kc in range(KC):
                nc.gpsimd.dma_start(out=w1_sb[0][kc][:], in_=w1[0, kc * PD:(kc + 1) * PD, :])

            # --- persistent constants / weights in SBUF ---
            misc128_sb = cpool.tile_from(misc128, forced_dma_engine=mybir.EngineType.Pool)
            wgt_sb = cpool.tile_from(wgt, forced_dma_engine=mybir.EngineType.Pool)
            bgt_sb = cpool.tile_from(bgt, forced_dma_engine=mybir.EngineType.Pool)
            misc8_sb = cpool.tile_from(misc8, forced_dma_engine=mybir.EngineType.Pool)
            ident_sb = cpool.tile_from(ident, forced_dma_engine=mybir.EngineType.Pool)
            # TensorScalarPtr (scalar-operand-from-AP) instructions can carry
            # only one sync wait, so scalar sources must be same-engine local:
            # stage DVE-consumed constants through a DVE copy and ACT-consumed
            # biases through an ACT copy.  After these copies each engine has
            # observed the const DMA sem once, so no later op re-waits on it.
            misc128L = cpool.tile([PD, 3 * S * KC + S], F32, name="misc128L")
            nc.vector.tensor_copy(out=misc128L[:], in_=misc128_sb[:])
            misc8L = cpool.tile([K, 2 * S * D + S], F32, name="misc8L")
            nc.vector.tensor_copy(out=misc8L[:], in_=misc8_sb[:])
            m128A = cpool.tile([PD, 3 * S * KC], F32, name="m128A")
            nc.scalar.copy(out=m128A[:], in_=misc128_sb[:, 0:3 * S * KC])
            b1tA = m128A[:, 0:S * KC]
            b2tA = m128A[:, 2 * S * KC:3 * S * KC]
            bgtA = cpool.tile([1, S], F32, name="bgtA")
            nc.scalar.copy(out=bgtA[:], in_=bgt_sb[:])
            bgtD = cpool.tile([1, S], F32, name="bgtD")
            nc.vector.tensor_copy(out=bgtD[:], in_=bgt_sb[:])
            b1t_sb = misc128L[:, 0:S * KC]
            sb1t_sb = misc128L[:, S * KC:2 * S * KC]
            b2t_sb = b2tA[:]
            slopet_sb = misc128L[:, 3 * S * KC:3 * S * KC + S]
            b1rep_sb = misc8L[:, 0:S * D]
            b2rep_sb = misc8L[:, S * D:2 * S * D]
            slope8_sb = misc8L[:, 2 * S * D:2 * S * D + S]

            # remaining weights (w1 slot 1, then w2) load behind the consts
            for kc in range(KC):
                nc.gpsimd.dma_start(out=w1_sb[1][kc][:], in_=w1[1, kc * PD:(kc + 1) * PD, :])
            for s in range(S):
                for kc in range(KC):
                    nc.gpsimd.dma_start(out=w2_sb[s][kc][:], in_=w2[s, kc * PD:(kc + 1) * PD, :])

            # ---------------- PASS A + per-b topk / pass B ----------------
            def topk_passB(b, lrow_b, cand):
                # ---- final top-k merge over the 64 per-tile candidates ----
                cand_v, cand_p = cand
                lrow = lrow_b[:]
                mx = spool.tile([S, K], F32, tag="mx")
                nc.vector.max(out=mx[:], in_=cand_v[:])
                masked = spool.tile([S, T * K], F32, tag="msk")
                nc.vector.match_replace(out=masked[:], in_to_replace=mx[:],
                                        in_values=cand_v[:], imm_value=-1e30)
                gtm = spool.tile([S, T * K], F32, tag="gtm")
                nc.vector.tensor_tensor(out=gtm[:], in0=cand_v[:], in1=masked[:],
                                        op=ALU.is_gt)
                mpk = spool.tile([S, T * K], F32, tag="mpk")
                nc.vector.tensor_tensor(out=mpk[:], in0=gtm[:], in1=cand_p[:],
                                        op=ALU.mult)
                # pk8: the 8 selected packs, descending by row index
                pk8 = spool.tile([S, K], F32, tag="pk8")
                nc.vector.max(out=pk8[:], in_=mpk[:])
                t3 = spool.tile([S, K], F32, tag="t3")
                nc.vector.tensor_scalar_mul(t3[:], pk8[:], 0.125)
                iu = spool.tile([S, K], U32, tag="iu")
                nc.vector.tensor_copy(out=iu[:], in_=t3[:])      # -> local idx+1
                tif2 = spool.tile([S, K], F32, tag="tif2")
                nc.vector.tensor_copy(out=tif2[:], in_=iu[:])
                # lhat + offset(-2) folded below; t5 = pk8 - 8*(idx+1)
                t5 = spool.tile([S, K], F32, tag="t5")
                nc.vector.scalar_tensor_tensor(
                    out=t5[:], in0=tif2[:], scalar=-8.0, in1=pk8[:],
                    op0=ALU.mult, op1=ALU.add)
                negvmax = spool.tile([S, 1], F32, tag="nvm")
                nc.vector.tensor_scalar_mul(negvmax[:], mx[:, 0:1], -1.0)
                expt = epool.tile([S, L], F32, tag="expt")
                zsum = spool.tile([S, 1], F32, tag="zsum")
                nc.scalar.activation(
                    out=expt[:], in_=lrow, func=AF.Exp,
                    bias=negvmax[:, 0:1], accum_out=zsum[:, 0:1],
                )
                recipz = spool.tile([S, 1], F32, tag="rz")
                nc.vector.reciprocal(recipz[:], zsum[:])
                # w8 = exp(lhat - vmax) * recipz, index-desc order (pass B
                # gathers rows in the same order, so any consistent order works)
                w8e = spool.tile([S, K], F32, tag="w8e")
                nc.vector.tensor_scalar(
                    out=w8e[:], in0=t5[:], scalar1=negvmax[:, 0:1],
                    scalar2=-2.0, op0=ALU.add, op1=ALU.add)
                nc.scalar.activation(out=w8e[:], in_=w8e[:], func=AF.Exp)
                w8 = spool.tile([S, K], F32, tag="w8")
                nc.vector.tensor_tensor(
                    out=w8[:], in0=w8e[:],
                    in1=recipz[:, 0:1].to_broadcast([S, K]), op=ALU.mult)
                # global row index = (local idx+1) - 1 + b*4096
                idxf = spool.tile([S, K], F32, tag="idxf")
                nc.vector.tensor_scalar_add(idxf[:], tif2[:], float(b * L - 1))
                # transpose idxf and w8 to [K, S] (rank on partitions)
                iw_ps = bps_pool.tile([K, 2 * S], F32, tag="bps", bufs=1)
                nc.tensor.transpose(iw_ps[:, 0:S], idxf[:], ident_sb[0:S, 0:S])
                nc.tensor.transpose(iw_ps[:, S:2 * S], w8[:], ident_sb[0:S, 0:S])
                iw_sb = spool.tile([K, 2 * S], F32, tag="iwsb")
                nc.vector.tensor_copy(out=iw_sb[:], in_=iw_ps[:])
                idxu = spool.tile([K, S], U32, tag="idxu")
                nc.vector.tensor_copy(out=idxu[:], in_=iw_sb[:, 0:S])

                # ---------------- PASS B: recompute top-8 rows ----------------
                for s in range(S):
                    q_sel = selpool.tile([K, D], F32, tag="qsel")
                    nc.gpsimd.indirect_dma_start(
                        out=q_sel[:], out_offset=None,
                        in_=qn_flat,
                        in_offset=bass.IndirectOffsetOnAxis(ap=idxu[:, s:s + 1], axis=0),
                    )
                    # q_selT chunks [128, 8] per kc
                    qst = selpool.tile([PD, KC * K], F32R, tag="qst")
                    for kc in range(KC):
                        t_ps = bps_pool.tile([PD, K], F32, tag="bps", bufs=1)
                        nc.tensor.transpose(
                            t_ps[:], q_sel[:, kc * PD:(kc + 1) * PD], ident_sb[0:K, 0:K])
                        nc.vector.tensor_copy(out=qst[:, kc * K:(kc + 1) * K], in_=t_ps[:])
                    # mm1 for selected rows: [8, 512]
                    hsel_ps = bps_pool.tile([K, D], F32, tag="bps", bufs=1)
                    for kc in range(KC):
                        nc.tensor.matmul(
                            hsel_ps[:],
                            lhsT=qst[:, kc * K:(kc + 1) * K],
                            rhs=w1_sb[s][kc][:],
                            start=(kc == 0), stop=(kc == KC - 1),
                        )
                    hsel = selpool.tile([K, D], F32, tag="hsel")
                    nc.vector.tensor_tensor(
                        out=hsel[:], in0=hsel_ps[:],
                        in1=b1rep_sb[:, s * D:(s + 1) * D], op=ALU.add)
                    nc.vector.scalar_tensor_tensor(
                        out=hsel[:], in0=hsel[:], scalar=slope8_sb[:, s:s + 1],
                        in1=hsel[:], op0=ALU.mult, op1=ALU.max)
                    # transpose hsel -> [128, 8] chunks
                    hst = selpool.tile([PD, KC * K], F32R, tag="hst")
                    for kc in range(KC):
                        t_ps = bps_pool.tile([PD, K], F32, tag="bps", bufs=1)
                        nc.tensor.transpose(
                            t_ps[:], hsel[:, kc * PD:(kc + 1) * PD], ident_sb[0:K, 0:K])
                        nc.vector.tensor_copy(out=hst[:, kc * K:(kc + 1) * K], in_=t_ps[:])
                    xsel_ps = bps_pool.tile([K, D], F32, tag="bps", bufs=1)
                    for kc in range(KC):
                        nc.tensor.matmul(
                            xsel_ps[:],
                            lhsT=hst[:, kc * K:(kc + 1) * K],
                            rhs=w2_sb[s][kc][:],
                            start=(kc == 0), stop=(kc == KC - 1),
                        )
                    xsel = selpool.tile([K, D], F32, tag="xsel")
                    nc.vector.tensor_tensor(
                        out=xsel[:], in0=xsel_ps[:],
                        in1=b2rep_sb[:, s * D:(s + 1) * D], op=ALU.add)
                    nc.scalar.activation(out=xsel[:], in_=xsel[:], func=AF.Tanh)
                    # combined[d] = sum_k w8[k] * xsel[k, d] -> [128, 1] per chunk
                    comb_ps = bps_pool.tile([PD, KC], F32, tag="bps", bufs=1)
                    for mc in range(KC):
                        nc.tensor.matmul(
                            comb_ps[:, mc:mc + 1],
                            lhsT=xsel[:, mc * PD:(mc + 1) * PD],
                            rhs=iw_sb[:, S + s:S + s + 1],
                            start=True, stop=True,
                        )
                    comb_sb = selpool.tile([PD, KC], F32, tag="combsb")
                    nc.vector.tensor_copy(out=comb_sb[:], in_=comb_ps[:])
                    nc.sync.dma_start(
                        out=out[s, :, b].rearrange("(mc p) -> p mc", p=PD),
                        in_=comb_sb[:],
                    )

            lrows = []
            cands = []
            # topk+passB for batch b is EMITTED inside batch b+1's tile loop
            # (after t==1) so its DVE/PE ops fill pipeline slack instead of
            # stalling the b->b+1 boundary; the last b runs after the loop.
            for b in range(B):
                lrow_b = lpool.tile([S, L], F32, tag="lrow", bufs=2)
                lrows.append(lrow_b)
                cand_v = lpool.tile([S, T * K], F32, tag="candv", bufs=2)
                cand_p = lpool.tile([S, T * K], F32, tag="candp", bufs=2)
                cands.append((cand_v, cand_p))
                for t in range(T):
                    # load qT tile: [128, kc x 512] (d on partitions, rows free)
                    qt_tile = qtpool.tile([PD, KC * 512], F32R, tag="qt")
                    for kc in range(KC):
                        nc.sync.dma_start(
                            out=qt_tile[:, kc * 512:(kc + 1) * 512],
                            in_=qt[b, kc * PD:(kc + 1) * PD,
                                   t * 512:(t + 1) * 512])

                    ht_tiles = []
                    # mm1 + leaky for both slots (interleaved for PE density)
                    for s in range(S):
                        ht = htpool.tile([PD, KC * 512], F32R, tag="ht")
                        ht_tiles.append(ht)
                        for mc in range(KC):
                            h_ps = hps_pool.tile([PD, 512], F32, tag="hps")
                            for kc in range(KC):
                                nc.tensor.matmul(
                                    h_ps[:],
                                    lhsT=w1_sb[s][kc][:, mc * PD:(mc + 1) * PD],
                                    rhs=qt_tile[:, kc * 512:(kc + 1) * 512],
                                    start=(kc == 0), stop=(kc == KC - 1),
                                )
                            # leaky: u = h + b1; out = max(slope*u, u).
                            # The bias add runs on ACT (Identity+bias from an
                            # ACT-local tile) to offload DVE; the DVE mult-max
                            # then carries a single ACT wait (TensorScalarPtr
                            # allows only one sync wait).
                            col = s * KC + mc
                            v = tmppool.tile([PD, 512], F32, tag="v")
                            nc.scalar.activation(
                                out=v[:], in_=h_ps[:], func=AF.Identity,
                                bias=b1tA[:, col:col + 1],
                            )
                            nc.vector.scalar_tensor_tensor(
                                out=ht[:, mc * 512:(mc + 1) * 512],
                                in0=v[:],
                                scalar=slopet_sb[:, s:s + 1],
                                in1=v[:],
                                op0=ALU.mult, op1=ALU.max,
                            )
                    xt_tiles = []
                    for s in range(S):
                        ht = ht_tiles[s]
                        xt = xtpool.tile([PD, KC * 512], F32R, tag="xt")
                        xt_tiles.append(xt)
                        for mc in range(KC):
                            x_ps = xps_pool.tile([PD, 512], F32, tag="xps")
                            for kc in range(KC):
                                nc.tensor.matmul(
                                    x_ps[:],
                                    lhsT=w2_sb[s][kc][:, mc * PD:(mc + 1) * PD],
                                    rhs=ht[:, kc * 512:(kc + 1) * 512],
                                    start=(kc == 0), stop=(kc == KC - 1),
                                )
                            col = s * KC + mc
                            nc.scalar.activation(
                                out=xt[:, mc * 512:(mc + 1) * 512], in_=x_ps[:],
                                func=AF.Tanh, bias=b2t_sb[:, col:col + 1],
                            )
                    # gate matvec per slot ([1, 512] psum each; PE out must
                    # start at partition 0). Engine ops can only address SBUF
                    # partitions 0/32/64/96, so stage the row at partition 0
                    # (with +bg) and DMA it into lrow_b's partition s.
                    for s in range(S):
                        g_ps = gps_pool.tile([1, 512], F32, tag="gps")
                        for kc in range(KC):
                            nc.tensor.matmul(
                                g_ps[:],
                                lhsT=wgt_sb[:, s * KC + kc:s * KC + kc + 1],
                                rhs=xt_tiles[s][:, kc * 512:(kc + 1) * 512],
                                start=(kc == 0), stop=(kc == KC - 1),
                            )
                        lstage = tmppool.tile([1, 512], F32, tag="lstage")
                        nc.vector.tensor_scalar(
                            out=lstage[:], in0=g_ps[:],
                            scalar1=bgtD[0:1, s:s + 1], scalar2=None,
                            op0=ALU.add,
                        )
                        nc.sync.dma_start(
                            out=lrow_b[s:s + 1, t * 512:(t + 1) * 512],
                            in_=lstage[:])

                    # hierarchical top-k, overlapped stage: per-tile top-8
                    # values + packed (index, clamped value) candidates.
                    # pack = 8*(local_idx+1) + (clamp(l, +-1.9) + 2): integer
                    # part recovers the index under truncation OR
                    # round-to-nearest; the fraction carries the logit to
                    # ~4e-3, used only for softmax weights (selection uses
                    # exact values in cand_v).
                    lslice = lrow_b[:, t * 512:(t + 1) * 512]
                    nc.vector.max(out=cand_v[:, t * K:(t + 1) * K], in_=lslice)
                    ti = spool.tile([S, K], U32, tag="ti")
                    nc.vector.max_index(out=ti[:], in_max=cand_v[:, t * K:(t + 1) * K],
                                        in_values=lslice)
                    tif = spool.tile([S, K], F32, tag="tif")
                    nc.vector.tensor_copy(out=tif[:], in_=ti[:])
                    t1 = spool.tile([S, K], F32, tag="t1")
                    nc.vector.tensor_scalar(
                        out=t1[:], in0=tif[:], scalar1=float(t * 512 + 1),
                        scalar2=8.0, op0=ALU.add, op1=ALU.mult)
                    t2 = spool.tile([S, K], F32, tag="t2")
                    nc.vector.tensor_scalar(
                        out=t2[:], in0=cand_v[:, t * K:(t + 1) * K],
                        scalar1=1.9, scalar2=-1.9, op0=ALU.min, op1=ALU.max)
                    nc.vector.scalar_tensor_tensor(
                        out=cand_p[:, t * K:(t + 1) * K], in0=t2[:], scalar=2.0,
                        in1=t1[:], op0=ALU.add, op1=ALU.add)

                    if t == 1 and b > 0:
                        topk_passB(b - 1, lrows[b - 1], cands[b - 1])


            topk_passB(B - 1, lrows[B - 1], cands[B - 1])

    nc.compile()  # Bacc passes: reg alloc, DCE, wait splitting (TRN2 1-wait rule)
    _PROGRAM_CACHE["nc"] = nc
    return nc


def _prep_in_maps(query, W1, b1, W2, b2, Wg, bg):
    query = np.ascontiguousarray(query, dtype=np.float32)
    qt = np.ascontiguousarray(query.transpose(0, 2, 1))
    slopes = (0.01 + 0.2 / N_MEM * np.arange(N_MEM, dtype=np.float32))
    ident = np.eye(PD, dtype=np.float32)
    in_maps = []
    for c in range(NCORES):
        sl = slice(S * c, S * (c + 1))
        b1c = np.asarray(b1[sl], np.float32)     # [S, D]
        b2c = np.asarray(b2[sl], np.float32)
        wgc = np.asarray(Wg[sl], np.float32)
        slc = slopes[sl]                          # [S]
        sb1c = slc[:, None] * b1c
        def t128(a):  # [S, D] -> [128, S*KC]
            return np.ascontiguousarray(
                a.reshape(S, KC, PD).transpose(2, 0, 1).reshape(PD, S * KC))
        misc128 = np.concatenate([
            t128(b1c), t128(sb1c), t128(b2c),
            np.broadcast_to(slc[None, :], (PD, S)),
        ], axis=1).astype(np.float32)
        misc8 = np.concatenate([
            np.broadcast_to(b1c.reshape(1, S * D), (K, S * D)),
            np.broadcast_to(b2c.reshape(1, S * D), (K, S * D)),
            np.broadcast_to(slc[None, :], (K, S)),
        ], axis=1).astype(np.float32)
        in_maps.append({
            "qt": qt,
            "qn": query,
            "w1": np.ascontiguousarray(W1[sl], np.float32),
            "w2": np.ascontiguousarray(W2[sl], np.float32),
            "misc128": np.ascontiguousarray(misc128),
            "wgt": t128(wgc),
            "bgt": np.asarray(bg[sl], np.float32).reshape(1, S),
            "misc8": np.ascontiguousarray(misc8),
            "ident": ident,
        })
    return in_maps


def kernel(query, W1, b1, W2, b2, Wg, bg, topk, **_ignored):
    assert int(topk) == K, f"kernel hardcodes topk={K}, got {topk}"
    nc = _build_program()
    in_maps = _prep_in_maps(
        np.asarray(query), np.asarray(W1), np.asarray(b1), np.asarray(W2),
        np.asarray(b2), np.asarray(Wg), np.asarray(bg))
    res = bass_utils.run_bass_kernel_spmd(nc, in_maps, core_ids=list(range(NCORES)))
    outs = res.results
    # outs[c]["out"] is [S, D, B]; assemble [B, N_MEM, D]
    m = np.empty((B, N_MEM, D), dtype=np.float32)
    for c in range(NCORES):
        oc = np.asarray(outs[c]["out"])  # [S, D, B]
        for s in range(S):
            m[:, S * c + s, :] = oc[s].T
    norm = np.maximum(np.linalg.norm(m.astype(np.float64), axis=-1, keepdims=True),
                      1e-12).astype(np.float32)
    return (m / norm).astype(np.float32)
